# revision 5
# baseline (speedup 1.0000x reference)
"""Trainium2 Bass kernel for PrivateGraphSAGE (2-layer PrivSAGEConv).

Push-mode distribution (8 NeuronCores, SPMD):
  - Nodes (x, noise, output) sharded across cores (6250 rows each).
  - Edges partitioned by SOURCE owner: each core computes partial messages
    msg_c[dst] = sum over its own-shard sources, for ALL destinations, using
    only its local clipped table (no AllGather!).
  - Per 128-dst tile: dma_gather pulls source rows from the core's own
    bf16 table; a one-hot built on DVE is the stationary matmul operand so
    the TensorEngine scatters the segment-sum into PSUM; PSUM partial tiles
    are staged wide and DMA'd to a [50176, 128] bf16 partial table.
  - A ReduceScatter (add) sums the 8 partial tables and leaves each core
    its own destination shard of the messages (output is only N/8 per core,
    which is far cheaper than an AllGather of the full table).
  - Epilogue per own tile: agg = xc + msg + noise; PE transpose + matmul
    with W.T; layer 1 fuses SELU + the next layer's clip and writes the
    layer-2 gather table; layer 2 writes the output.
"""

import contextlib

import numpy as np

import concourse.bacc as bacc
import concourse.bass as bass
import concourse.mybir as mybir
import concourse.tile as tile
from concourse.bass_utils import run_bass_kernel_spmd

F32 = mybir.dt.float32
BF16 = mybir.dt.bfloat16
I16 = mybir.dt.int16

SELU_LAM = 1.0507009873554804934193349852946
SELU_ALPHA = 1.6732632423543772848170429916717

N_NODES = 50000
NCORES = 8


# ---------------------------------------------------------------------------
# Host-side preprocessing
# ---------------------------------------------------------------------------

def _preprocess(src, dst, n_nodes=N_NODES, ncores=NCORES):
    """Partition edges by source owner; bucket by global 128-dst tile; pad
    each bucket to G[tile]*128 edges with G uniform across cores (SPMD)."""
    S = -(-n_nodes // ncores)          # 6250 rows per shard
    NT = -(-S // 128)                  # 49 tiles per shard
    SPAD = NT * 128                    # 6272
    GT = ncores * NT                   # 392 global dst tiles

    src = np.asarray(src, np.int64)
    dst = np.asarray(dst, np.int64)
    c = src // S                       # owner core (src < 50000 -> c <= 7)
    lsrc = src - c * S                 # local row in owner's table
    cd = dst // S
    ld = dst - cd * S
    gt = cd * NT + ld // 128           # global padded dst tile
    rel = ld % 128

    key = c * GT + gt
    counts = np.bincount(key, minlength=ncores * GT).reshape(ncores, GT)
    G = np.maximum(1, -(-counts.max(axis=0) // 128))       # [GT]
    off = np.concatenate([[0], np.cumsum(G)[:-1]]).astype(np.int64)
    g_tot = int(G.sum())
    e_pad = g_tot * 128

    order = np.argsort(key, kind="stable")
    key_s = key[order]
    lsrc_s = lsrc[order]
    rel_s = rel[order]
    run_start = np.concatenate(
        [[0], np.cumsum(np.bincount(key_s, minlength=ncores * GT))[:-1]])
    within = np.arange(len(key_s)) - run_start[key_s]
    slot = off[key_s % GT] * 128 + within
    cc = key_s // GT

    idxp = np.zeros((ncores, e_pad), np.int16)
    tagp = np.full((ncores, e_pad), -1.0, np.float32)
    idxp[cc, slot] = lsrc_s.astype(np.int16)
    tagp[cc, slot] = rel_s

    # int16 gather indices: idx j of the flat edge array lives at
    # [j % 16, j // 16]; replicated across all eight 16-row bands because
    # different Q7 ucode versions read different bands.
    idx16 = idxp.reshape(ncores, e_pad // 16, 16).transpose(0, 2, 1)
    idx16 = np.ascontiguousarray(np.tile(idx16, (1, 8, 1)))

    # dst tags, INTERLEAVED per tile: for tile t (G groups at off[t]), the
    # tag of (group g, edge slot e) sits at column off[t] + e... no — tags
    # stay per-group columns [off[t]+g], but stored bf16 for DVE fast modes.
    import ml_dtypes
    drel = np.ascontiguousarray(
        tagp.reshape(ncores, g_tot, 128).transpose(0, 2, 1)
    ).astype(ml_dtypes.bfloat16)

    # runs of consecutive tiles with equal G (capped so R*G <= 16)
    runs = []
    t = 0
    while t < GT:
        Gv = int(G[t])
        R = 1
        while (t + R < GT and int(G[t + R]) == Gv and (R + 1) * Gv <= 16
               and R < 8):
            R += 1
        runs.append((t, R, Gv))
        t += R

    meta = dict(n_nodes=n_nodes, ncores=ncores, S=S, NT=NT, SPAD=SPAD,
                GT=GT, G=G, off=off, g_tot=g_tot, runs=runs)
    return meta, idx16, drel


# ---------------------------------------------------------------------------
# Device program
# ---------------------------------------------------------------------------

def _onehot_run(nc, oh, drel_sb, ot, Gv, R, iotar_sb):
    """One DVE op building interleaved one-hots for R consecutive tiles of
    G=Gv groups each: oh[e, r*Gv*128 + d*Gv + g] = (tag[e, ot+r*Gv+g] == d).
    All operands bf16/packed/SBUF so DVE fast modes can kick in."""
    a = oh[:]
    o4 = bass.AP(a.tensor, a.offset,
                 [list(a.ap[0]), [Gv * 128, R], [Gv, 128], [1, Gv]])
    d = drel_sb[:]
    d4 = bass.AP(d.tensor, d.offset + ot,
                 [list(d.ap[0]), [Gv, R], [0, 128], [1, Gv]])
    i = iotar_sb[:]
    i4 = bass.AP(i.tensor, i.offset,
                 [list(i.ap[0]), [0, R], [Gv, 128], [1, Gv]])
    nc.vector.tensor_tensor(o4, d4, i4, op=mybir.AluOpType.is_equal)


def _oh_col(oh, r, Gv, j):
    """lhsT AP of tile r's group j inside an interleaved one-hot run tile:
    columns r*Gv*128 + d*Gv + j for d in 0..127."""
    a = oh[:]
    return bass.AP(a.tensor, a.offset + r * Gv * 128 + j,
                   [list(a.ap[0]), [Gv, 128]])


def _wide_dram(t, r0, nrows):
    """DRAM rows [r0, r0+nrows*128) as [128, nrows, 128]."""
    return t[r0 * 128:(r0 + nrows) * 128, :].rearrange("(g p) f -> p g f", p=128)


def _wide_sbuf(t, nrows):
    return t[:, :nrows * 128].rearrange("p (g f) -> p g f", f=128)


def _build_program(meta, with_b):
    m = meta
    S, NT, SPAD, GT = m["S"], m["NT"], m["SPAD"], m["GT"]
    G, off, g_tot = m["G"], m["off"], m["g_tot"]
    ncores = m["ncores"]
    NTAB = ncores * SPAD
    Gmax = int(G.max())
    rg = [list(range(ncores))]

    nc = bacc.Bacc(None, target_bir_lowering=False)

    xs = nc.declare_dram_parameter("xs", [SPAD, 128], F32, isOutput=False)
    n1s = nc.declare_dram_parameter("n1s", [SPAD, 128], F32, isOutput=False)
    n2s = nc.declare_dram_parameter("n2s", [SPAD, 128], F32, isOutput=False)
    w1t = nc.declare_dram_parameter("w1t", [128, 128], F32, isOutput=False)
    w2t = nc.declare_dram_parameter("w2t", [128, 128], F32, isOutput=False)
    idxp = nc.declare_dram_parameter("idx", [128, g_tot * 8], I16, isOutput=False)
    drel = nc.declare_dram_parameter("dstrel", [128, g_tot], BF16, isOutput=False)
    gvals = sorted({gv for _, _, gv in m["runs"]})
    iotarp = {gv: nc.declare_dram_parameter(f"iotar{gv}", [128, gv * 128],
                                            BF16, isOutput=False)
              for gv in gvals}
    identp = nc.declare_dram_parameter("ident", [128, 128], F32, isOutput=False)
    if with_b:
        b1p = nc.declare_dram_parameter("b1r", [1, 128], F32, isOutput=False)
        b2p = nc.declare_dram_parameter("b2r", [1, 128], F32, isOutput=False)
    outp = nc.declare_dram_parameter("out", [SPAD, 128], F32, isOutput=True)

    xtab = nc.dram_tensor("xtab", [SPAD, 128], BF16)
    htab = nc.dram_tensor("htab", [SPAD, 128], BF16)
    msgp1 = nc.dram_tensor("msgp1", [NTAB, 128], BF16)
    msgp2 = nc.dram_tensor("msgp2", [NTAB, 128], BF16)
    msgs1 = nc.dram_tensor("msgs1", [SPAD, 128], BF16)
    msgs2 = nc.dram_tensor("msgs2", [SPAD, 128], BF16)

    mult = mybir.AluOpType.mult
    add = mybir.AluOpType.add
    Act = mybir.ActivationFunctionType

    from concourse.library_config import mlp
    nc.gpsimd.load_library(mlp)

    with tile.TileContext(nc) as tc:
        with contextlib.ExitStack() as ctx:
            cpool = ctx.enter_context(tc.tile_pool(name="const", bufs=1))
            xin = ctx.enter_context(tc.tile_pool(name="xin", bufs=3))
            pa = ctx.enter_context(tc.tile_pool(name="pa", bufs=4))
            pa1 = ctx.enter_context(tc.tile_pool(name="pa1", bufs=4))
            selfp = ctx.enter_context(tc.tile_pool(name="selfp", bufs=1))
            stgb = ctx.enter_context(tc.tile_pool(name="stgb", bufs=3))
            stgp = ctx.enter_context(tc.tile_pool(name="stgp", bufs=3))
            gp = ctx.enter_context(tc.tile_pool(name="gather", bufs=8))
            ohp = ctx.enter_context(tc.tile_pool(name="onehot", bufs=4))
            mrp = ctx.enter_context(tc.tile_pool(name="mread", bufs=2))
            nrp = ctx.enter_context(tc.tile_pool(name="nread", bufs=2))
            ep = ctx.enter_context(tc.tile_pool(name="epil", bufs=4))
            eps = ctx.enter_context(tc.tile_pool(name="epilsc", bufs=4))
            psA = ctx.enter_context(tc.tile_pool(name="psA", bufs=4, space="PSUM"))
            psT = ctx.enter_context(tc.tile_pool(name="psT", bufs=2, space="PSUM"))
            psO = ctx.enter_context(tc.tile_pool(name="psO", bufs=2, space="PSUM"))

            # ---- constants ------------------------------------------------
            w1t_sb = cpool.tile([128, 128], F32, tag="w1t")
            nc.sync.dma_start(w1t_sb[:], w1t[:])
            w2t_sb = cpool.tile([128, 128], F32, tag="w2t")
            nc.sync.dma_start(w2t_sb[:], w2t[:])
            iotar_sb = {}
            for gv in gvals:
                tl = cpool.tile([128, gv * 128], BF16, tag=f"iotar{gv}")
                nc.sync.dma_start(tl[:], iotarp[gv][:])
                iotar_sb[gv] = tl
            ident_sb = cpool.tile([128, 128], F32, tag="ident")
            nc.sync.dma_start(ident_sb[:], identp[:])
            idx_sb = cpool.tile([128, g_tot * 8], I16, tag="idx")
            nc.sync.dma_start(idx_sb[:], idxp[:])
            drel_sb = cpool.tile([128, g_tot], BF16, tag="drel")
            nc.sync.dma_start(drel_sb[:], drel[:])
            if with_b:
                b1_sb = cpool.tile([1, 128], F32, tag="b1")
                nc.sync.dma_start(b1_sb[:], b1p[:])
                b2_sb = cpool.tile([1, 128], F32, tag="b2")
                nc.sync.dma_start(b2_sb[:], b2p[:])
                ones_sb = cpool.tile([1, 128], F32, tag="ones")
                nc.gpsimd.memset(ones_sb[:], 1.0)
            lnal_sb = cpool.tile([128, 1], F32, tag="lnal")
            nc.gpsimd.memset(lnal_sb[:], float(np.log(SELU_ALPHA)))

            xcs = [None] * NT     # persistent f32 clipped x tiles
            hcs = [None] * NT     # persistent f32 clipped h tiles

            # ---- phase A: clip own x shard --------------------------------
            stw = None
            ws = 0
            for t in range(NT):
                if t % 4 == 0:
                    nb = min(4, NT - t)
                    xw = xin.tile([128, 512], F32, tag="xw")
                    nc.sync.dma_start(_wide_sbuf(xw, nb), _wide_dram(xs, t, nb))
                xt = xw[:, (t % 4) * 128:(t % 4 + 1) * 128]
                sq = pa.tile([128, 128], F32, tag="sq")
                ss = pa1.tile([128, 1], F32, tag="ss")
                nc.scalar.activation(sq[:], xt, Act.Square, accum_out=ss[:])
                nrm = pa1.tile([128, 1], F32, tag="nrm")
                nc.scalar.activation(nrm[:], ss[:], Act.Sqrt)
                dd = pa1.tile([128, 1], F32, tag="dd")
                nc.vector.tensor_scalar_max(dd[:], nrm[:], 1.0)
                sc = pa1.tile([128, 1], F32, tag="sc")
                nc.vector.reciprocal(sc[:], dd[:])
                xc_t = selfp.tile([128, 128], F32, tag=f"xc{t}")
                nc.vector.tensor_tensor(xc_t[:], xt, sc[:].to_broadcast([128, 128]),
                                        op=mult)
                xcs[t] = xc_t
                if t % 8 == 0:
                    stw = stgb.tile([128, 1024], BF16, tag="stx")
                    ws = t
                nc.scalar.activation(stw[:, (t % 8) * 128:(t % 8 + 1) * 128],
                                     xc_t[:], Act.Copy)
                if t % 8 == 7 or t == NT - 1:
                    nb = t - ws + 1
                    nc.sync.dma_start(_wide_dram(xtab, ws, nb), _wide_sbuf(stw, nb))

            # ---- edge phase ----------------------------------------------
            def edge_phase(tab, msgp, lname):
                ncalls = (g_tot + 7) // 8
                gts = []
                for k in range(ncalls):
                    ng = min(8, g_tot - k * 8)
                    gtile = gp.tile([128, 1024], BF16, tag=f"g{lname}")
                    nc.gpsimd.dma_gather(
                        gtile[:, :ng * 128].rearrange("p (g e) -> p g e", e=128),
                        tab[:, :],
                        idx_sb[:, k * 64:k * 64 + ng * 8],
                        ng * 128, ng * 128, 128)
                    gts.append(gtile)
                stw = pag = None
                ws = p4 = 0
                for (t0, R, Gv) in m["runs"]:
                    oh = ohp.tile([128, 2048], BF16, tag="oh")
                    _onehot_run(nc, oh, drel_sb, int(off[t0]), Gv, R,
                                iotar_sb[Gv])
                    for r in range(R):
                        t = t0 + r
                        ot = int(off[t])
                        if t % 8 == 0:
                            stw = stgp.tile([128, 1024], BF16,
                                            tag=f"stp{lname}")
                            ws = t
                        if t % 4 == 0:
                            pag = psA.tile([128, 512], F32, tag="pag")
                            p4 = t
                        col = (t % 4) * 128
                        for j in range(Gv):
                            k, s = divmod(ot + j, 8)
                            nc.tensor.matmul(
                                pag[:, col:col + 128],
                                lhsT=_oh_col(oh, r, Gv, j),
                                rhs=gts[k][:, s * 128:(s + 1) * 128],
                                start=(j == 0), stop=(j == Gv - 1))
                        if t % 4 == 3 or t == GT - 1:
                            nbank = t - p4 + 1
                            dsl = stw[:, (p4 - ws) * 128:
                                      (p4 - ws + nbank) * 128]
                            if (t // 4) % 2 == 0:
                                nc.vector.tensor_copy(dsl, pag[:, :nbank * 128])
                            else:
                                nc.scalar.activation(dsl, pag[:, :nbank * 128],
                                                     Act.Copy)
                        if t % 8 == 7 or t == GT - 1:
                            nb = t - ws + 1
                            nc.sync.dma_start(_wide_dram(msgp, ws, nb),
                                              _wide_sbuf(stw, nb))

            # ---- epilogue --------------------------------------------------
            def epilogue(msgs, noise, wt_sb, b_sb, self_tiles, out_tiles,
                         selu, lname):
                mw = nw = None
                for t in range(NT):
                    if t % 8 == 0:
                        nb = min(8, NT - t)
                        mw = mrp.tile([128, 1024], BF16, tag=f"mw{lname}")
                        nc.sync.dma_start(_wide_sbuf(mw, nb),
                                          _wide_dram(msgs, t, nb))
                    if t % 4 == 0:
                        nb = min(4, NT - t)
                        nw = nrp.tile([128, 512], F32, tag=f"nw{lname}")
                        nc.sync.dma_start(_wide_sbuf(nw, nb),
                                          _wide_dram(noise, t, nb))
                    a1 = ep.tile([128, 128], F32, tag="a1")
                    nc.vector.tensor_tensor(
                        a1[:], self_tiles[t][:],
                        mw[:, (t % 8) * 128:(t % 8 + 1) * 128], op=add)
                    agg = ep.tile([128, 128], F32, tag="agg")
                    nc.vector.tensor_tensor(
                        agg[:], a1[:],
                        nw[:, (t % 4) * 128:(t % 4 + 1) * 128], op=add)
                    pt = psT.tile([128, 128], F32, tag="pt")
                    nc.tensor.transpose(pt[:], agg[:], ident_sb[:])
                    agT = ep.tile([128, 128], F32, tag="agT")
                    if t % 2 == 0:
                        nc.vector.tensor_copy(agT[:], pt[:])
                    else:
                        nc.scalar.activation(agT[:], pt[:], Act.Copy)
                    po = psO.tile([128, 128], F32, tag="po")
                    if b_sb is not None:
                        nc.tensor.matmul(po[:], lhsT=ones_sb[:], rhs=b_sb[:],
                                         start=True, stop=False)
                        nc.tensor.matmul(po[:], lhsT=agT[:], rhs=wt_sb[:],
                                         start=False, stop=True)
                    else:
                        nc.tensor.matmul(po[:], lhsT=agT[:], rhs=wt_sb[:],
                                         start=True, stop=True)
                    if t % 8 == 0:
                        ostw = (stgb if selu else stgp).tile(
                            [128, 1024], BF16 if selu else F32,
                            tag=f"so{lname}")
                        ws = t
                    if selu:
                        t0 = ep.tile([128, 128], F32, tag="t0")
                        nc.vector.tensor_scalar_min(t0[:], po[:], 0.0)
                        e_ = ep.tile([128, 128], F32, tag="e_")
                        nc.scalar.activation(e_[:], t0[:], Act.Exp,
                                             bias=lnal_sb[:])
                        m_ = ep.tile([128, 128], F32, tag="m_")
                        nc.vector.tensor_scalar_max(m_[:], po[:], 0.0)
                        u_ = ep.tile([128, 128], F32, tag="u_")
                        nc.vector.tensor_tensor(u_[:], m_[:], e_[:], op=add)
                        hh = ep.tile([128, 128], F32, tag="hh")
                        nc.scalar.activation(hh[:], u_[:], Act.Copy,
                                             bias=-SELU_LAM * SELU_ALPHA,
                                             scale=SELU_LAM)
                        sq2 = ep.tile([128, 128], F32, tag="sq2")
                        ss2 = eps.tile([128, 1], F32, tag="ss2")
                        nc.scalar.activation(sq2[:], hh[:], Act.Square,
                                             accum_out=ss2[:])
                        nr2 = eps.tile([128, 1], F32, tag="nr2")
                        nc.scalar.activation(nr2[:], ss2[:], Act.Sqrt)
                        dd2 = eps.tile([128, 1], F32, tag="dd2")
                        nc.vector.tensor_scalar_max(dd2[:], nr2[:], 1.0)
                        sc2 = eps.tile([128, 1], F32, tag="sc2")
                        nc.vector.reciprocal(sc2[:], dd2[:])
                        hc_t = selfp.tile([128, 128], F32, tag=f"hc{t}")
                        nc.vector.tensor_tensor(
                            hc_t[:], hh[:], sc2[:].to_broadcast([128, 128]),
                            op=mult)
                        out_tiles[t] = hc_t
                        nc.scalar.activation(
                            ostw[:, (t % 8) * 128:(t % 8 + 1) * 128],
                            hc_t[:], Act.Copy)
                        if t % 8 == 7 or t == NT - 1:
                            nb = t - ws + 1
                            nc.sync.dma_start(_wide_dram(htab, ws, nb),
                                              _wide_sbuf(ostw, nb))
                    else:
                        nc.scalar.activation(
                            ostw[:, (t % 8) * 128:(t % 8 + 1) * 128],
                            po[:], Act.Copy)
                        if t % 8 == 7 or t == NT - 1:
                            nb = t - ws + 1
                            nc.sync.dma_start(_wide_dram(outp, ws, nb),
                                              _wide_sbuf(ostw, nb))

            # ---- layer 1 ---------------------------------------------------
            edge_phase(xtab, msgp1, "a")
            nc.gpsimd.collective_compute(
                "ReduceScatter", add, ins=[msgp1[:, :]], outs=[msgs1[:, :]],
                replica_groups=rg)
            epilogue(msgs1, n1s, w1t_sb, b1_sb if with_b else None,
                     xcs, hcs, selu=True, lname="a")

            # ---- layer 2 ---------------------------------------------------
            edge_phase(htab, msgp2, "b")
            nc.gpsimd.collective_compute(
                "ReduceScatter", add, ins=[msgp2[:, :]], outs=[msgs2[:, :]],
                replica_groups=rg)
            epilogue(msgs2, n2s, w2t_sb, b2_sb if with_b else None,
                     hcs, [None] * NT, selu=False, lname="b")

    nc.compile()
    return nc


# ---------------------------------------------------------------------------
# Entry point
# ---------------------------------------------------------------------------

def _make_inmaps(inputs, meta, idx16, drel, with_b):
    S, SPAD, ncores = meta["S"], meta["SPAD"], meta["ncores"]
    n_nodes = meta["n_nodes"]
    x = np.ascontiguousarray(np.asarray(inputs["x"], np.float32))
    w1 = np.asarray(inputs["W1"], np.float32)
    w2 = np.asarray(inputs["W2"], np.float32)
    no1 = np.asarray(inputs["noise1"], np.float32)
    no2 = np.asarray(inputs["noise2"], np.float32)

    def shard(arr, c):
        lo = c * S
        hi = min(lo + S, n_nodes)
        out = np.zeros((SPAD, 128), np.float32)
        out[:hi - lo] = arr[lo:hi]
        return out

    import ml_dtypes
    ident = np.eye(128, dtype=np.float32)
    iotar = {}
    for _, _, gv in meta["runs"]:
        if gv not in iotar:
            iotar[gv] = np.ascontiguousarray(np.tile(
                (np.arange(gv * 128) // gv).astype(ml_dtypes.bfloat16),
                (128, 1)))
    in_maps = []
    for c in range(ncores):
        im = dict(
            xs=shard(x, c), n1s=shard(no1, c), n2s=shard(no2, c),
            w1t=np.ascontiguousarray(w1.T), w2t=np.ascontiguousarray(w2.T),
            idx=idx16[c], dstrel=drel[c], ident=ident,
            **{f"iotar{gv}": arr for gv, arr in iotar.items()},
        )
        if with_b:
            im["b1r"] = np.asarray(inputs["b1"], np.float32).reshape(1, 128)
            im["b2r"] = np.asarray(inputs["b2"], np.float32).reshape(1, 128)
        in_maps.append(im)
    return in_maps


def _run(inputs, ncores=NCORES, sim=False, trace=False):
    ei = np.asarray(inputs["edge_index"], np.int64)
    n_nodes = int(np.asarray(inputs["x"]).shape[0])
    meta, idx16, drel = _preprocess(ei[0], ei[1], n_nodes, ncores)
    with_b = bool(np.any(np.asarray(inputs["b1"])) or
                  np.any(np.asarray(inputs["b2"])))
    nc = _build_program(meta, with_b)
    in_maps = _make_inmaps(inputs, meta, idx16, drel, with_b)
    S, SPAD = meta["S"], meta["SPAD"]

    if sim:
        from concourse.bass_interp import MultiCoreSim
        msim = MultiCoreSim(nc, ncores, trace=trace)
        for c in range(ncores):
            for k, v in in_maps[c].items():
                msim.cores[c].tensor(k)[:] = v
        msim.simulate()
        results = [{"out": np.array(msim.cores[c].tensor("out"))}
                   for c in range(ncores)]
        res = msim
    else:
        res = run_bass_kernel_spmd(nc, in_maps, core_ids=list(range(ncores)),
                                   trace=trace)
        results = res.results

    parts = []
    for c in range(ncores):
        lo = c * S
        hi = min(lo + S, n_nodes)
        parts.append(results[c]["out"][:hi - lo])
    out = np.concatenate(parts, axis=0).astype(np.float32)
    return out, res


def kernel(**inputs) -> np.ndarray:
    out, _ = _run(inputs, ncores=NCORES, sim=False)
    return out


# revision 11
# speedup vs baseline: 1.9333x; 1.9333x over previous
"""Trainium2 Bass kernel for PrivateGraphSAGE (2-layer PrivSAGEConv).

Push-mode distribution (8 NeuronCores, SPMD):
  - Nodes (x, noise, output) sharded across cores (6250 rows each).
  - Edges partitioned by SOURCE owner: each core computes partial messages
    for ALL destinations using only its local clipped table (no AllGather).
  - Per 128-dst tile: dma_gather pulls source rows from the core's own bf16
    table; interleaved one-hots (bf16, built on DVE in one op per run of
    equal-G tiles) are the stationary matmul operands so the TensorEngine
    scatters segment-sums into wide [128,512] PSUM banks; one wide copy per
    bank stages partials which are written 8 tiles per DMA into a
    [50176,128] bf16 partial table.
  - A ReduceScatter(add) leaves each core its own destination shard of the
    summed messages (output N/8 -> ~55us vs ~250us for an AllGather).
  - Epilogues run in wide 4-tile blocks, two passes (SELU -> norms -> clip
    scale), with ACT pinned to the exp-family function set and the scalar
    chain on DVE to avoid activation-table reloads.
"""

import contextlib

import numpy as np

import concourse.bacc as bacc
import concourse.bass as bass
import concourse.mybir as mybir
import concourse.tile as tile
from concourse.bass_utils import run_bass_kernel_spmd

F32 = mybir.dt.float32
BF16 = mybir.dt.bfloat16
F16 = mybir.dt.bfloat16  # fp16 collectives unproven on NRT; bf16 is HW-validated
I16 = mybir.dt.int16

LAM = 1.0507009873554804934193349852946
ALPHA = 1.6732632423543772848170429916717

N_NODES = 50000
NCORES = 8


# ---------------------------------------------------------------------------
# Host-side preprocessing
# ---------------------------------------------------------------------------

def _preprocess(src, dst, n_nodes=N_NODES, ncores=NCORES):
    """Partition edges by source owner; bucket by global 128-dst tile; pad
    each bucket to G[tile]*128 edges with G uniform across cores (SPMD)."""
    S = -(-n_nodes // ncores)
    NT = -(-S // 128)
    SPAD = NT * 128
    GT = ncores * NT

    src = np.asarray(src, np.int64)
    dst = np.asarray(dst, np.int64)
    c = src // S
    lsrc = src - c * S
    cd = dst // S
    ld = dst - cd * S
    gt = cd * NT + ld // 128
    rel = ld % 128

    key = c * GT + gt
    counts = np.bincount(key, minlength=ncores * GT).reshape(ncores, GT)
    G = np.maximum(1, -(-counts.max(axis=0) // 128))
    off = np.concatenate([[0], np.cumsum(G)[:-1]]).astype(np.int64)
    g_tot = int(G.sum())
    e_pad = g_tot * 128

    order = np.argsort(key, kind="stable")
    key_s = key[order]
    lsrc_s = lsrc[order]
    rel_s = rel[order]
    run_start = np.concatenate(
        [[0], np.cumsum(np.bincount(key_s, minlength=ncores * GT))[:-1]])
    within = np.arange(len(key_s)) - run_start[key_s]
    slot = off[key_s % GT] * 128 + within
    cc = key_s // GT

    idxp = np.zeros((ncores, e_pad), np.int16)
    tagp = np.full((ncores, e_pad), -1.0, np.float32)
    idxp[cc, slot] = lsrc_s.astype(np.int16)
    tagp[cc, slot] = rel_s

    idx16 = idxp.reshape(ncores, e_pad // 16, 16).transpose(0, 2, 1)
    idx16 = np.ascontiguousarray(np.tile(idx16, (1, 8, 1)))

    import ml_dtypes
    drel = np.ascontiguousarray(
        tagp.reshape(ncores, g_tot, 128).transpose(0, 2, 1)
    ).astype(ml_dtypes.bfloat16)

    runs = []
    t = 0
    while t < GT:
        Gv = int(G[t])
        R = 1
        while (t + R < GT and int(G[t + R]) == Gv and (R + 1) * Gv <= 16
               and R < 8):
            R += 1
        runs.append((t, R, Gv))
        t += R

    meta = dict(n_nodes=n_nodes, ncores=ncores, S=S, NT=NT, SPAD=SPAD,
                GT=GT, G=G, off=off, g_tot=g_tot, runs=runs)
    return meta, idx16, drel


# ---------------------------------------------------------------------------
# Device program helpers
# ---------------------------------------------------------------------------

def _onehot_run(nc, oh, drel_sb, ot, Gv, R, iotar_sb):
    """One DVE op building interleaved one-hots for R consecutive tiles of
    G=Gv groups each: oh[e, r*Gv*128 + d*Gv + g] = (tag[e, ot+r*Gv+g] == d)."""
    a = oh[:]
    o4 = bass.AP(a.tensor, a.offset,
                 [list(a.ap[0]), [Gv * 128, R], [Gv, 128], [1, Gv]])
    d = drel_sb[:]
    d4 = bass.AP(d.tensor, d.offset + ot,
                 [list(d.ap[0]), [Gv, R], [0, 128], [1, Gv]])
    i = iotar_sb[:]
    i4 = bass.AP(i.tensor, i.offset,
                 [list(i.ap[0]), [0, R], [Gv, 128], [1, Gv]])
    nc.vector.tensor_tensor(o4, d4, i4, op=mybir.AluOpType.is_equal)


def _oh_col(oh, r, Gv, j):
    a = oh[:]
    return bass.AP(a.tensor, a.offset + r * Gv * 128 + j,
                   [list(a.ap[0]), [Gv, 128]])


def _wide_dram(t, r0, nrows):
    return t[r0 * 128:(r0 + nrows) * 128, :].rearrange("(g p) f -> p g f", p=128)


def _wide_sbuf(t, nrows, col0=0):
    return t[:, col0 * 128:(col0 + nrows) * 128].rearrange(
        "p (g f) -> p g f", f=128)


def _build_program(meta, with_b):
    m = meta
    S, NT, SPAD, GT = m["S"], m["NT"], m["SPAD"], m["GT"]
    G, off, g_tot = m["G"], m["off"], m["g_tot"]
    ncores = m["ncores"]
    NTAB = ncores * SPAD
    NBLK = (NT + 3) // 4
    rg = [list(range(ncores))]

    nc = bacc.Bacc(None, target_bir_lowering=False)

    xs = nc.declare_dram_parameter("xs", [SPAD, 128], F32, isOutput=False)
    n1s = nc.declare_dram_parameter("n1s", [SPAD, 128], F32, isOutput=False)
    n2s = nc.declare_dram_parameter("n2s", [SPAD, 128], F32, isOutput=False)
    w1t = nc.declare_dram_parameter("w1t", [128, 128], F32, isOutput=False)
    w2t = nc.declare_dram_parameter("w2t", [128, 128], F32, isOutput=False)
    idxp = nc.declare_dram_parameter("idx", [128, g_tot * 8], I16, isOutput=False)
    drel = nc.declare_dram_parameter("dstrel", [128, g_tot], BF16, isOutput=False)
    gvals = sorted({gv for _, _, gv in m["runs"]})
    iotarp = {gv: nc.declare_dram_parameter(f"iotar{gv}", [128, gv * 128],
                                            BF16, isOutput=False)
              for gv in gvals}
    identp = nc.declare_dram_parameter("ident", [128, 128], F32, isOutput=False)
    if with_b:
        b1p = nc.declare_dram_parameter("b1r", [1, 128], F32, isOutput=False)
        b2p = nc.declare_dram_parameter("b2r", [1, 128], F32, isOutput=False)
    outp = nc.declare_dram_parameter("out", [SPAD, 128], F32, isOutput=True)

    xtab = nc.dram_tensor("xtab", [SPAD, 128], BF16)
    htab = nc.dram_tensor("htab", [SPAD, 128], BF16)
    msgp1 = nc.dram_tensor("msgp1", [NTAB, 128], F16)
    msgp2 = nc.dram_tensor("msgp2", [NTAB, 128], F16)
    msgs1 = nc.dram_tensor("msgs1", [SPAD, 128], F16)
    msgs2 = nc.dram_tensor("msgs2", [SPAD, 128], F16)

    mult = mybir.AluOpType.mult
    add = mybir.AluOpType.add
    sub = mybir.AluOpType.subtract
    Act = mybir.ActivationFunctionType

    from concourse.library_config import mlp
    nc.gpsimd.load_library(mlp)

    with tile.TileContext(nc) as tc:
        with contextlib.ExitStack() as ctx:
            cpool = ctx.enter_context(tc.tile_pool(name="const", bufs=1))
            xin = ctx.enter_context(tc.tile_pool(name="xin", bufs=3))
            pa = ctx.enter_context(tc.tile_pool(name="pa", bufs=2))
            selfp = ctx.enter_context(tc.tile_pool(name="selfp", bufs=1))
            hhp = ctx.enter_context(tc.tile_pool(name="hhp", bufs=1))
            stgb = ctx.enter_context(tc.tile_pool(name="stgb", bufs=2))
            stgp = ctx.enter_context(tc.tile_pool(name="stgp", bufs=2))
            gp = ctx.enter_context(tc.tile_pool(name="gather", bufs=6))
            ohp = ctx.enter_context(tc.tile_pool(name="onehot", bufs=3))
            mrp = ctx.enter_context(tc.tile_pool(name="mread", bufs=2))
            nrp = ctx.enter_context(tc.tile_pool(name="nread", bufs=2))
            ep = ctx.enter_context(tc.tile_pool(name="epil", bufs=3))
            eps = ctx.enter_context(tc.tile_pool(name="epilsc", bufs=1))
            psA = ctx.enter_context(tc.tile_pool(name="psA", bufs=4, space="PSUM"))
            psT = ctx.enter_context(tc.tile_pool(name="psT", bufs=2, space="PSUM"))
            psO = ctx.enter_context(tc.tile_pool(name="psO", bufs=2, space="PSUM"))

            # ---- constants ------------------------------------------------
            w1t_sb = cpool.tile([128, 128], F32, tag="w1t")
            nc.sync.dma_start(w1t_sb[:], w1t[:])
            w2t_sb = cpool.tile([128, 128], F32, tag="w2t")
            nc.sync.dma_start(w2t_sb[:], w2t[:])
            iotar_sb = {}
            for gv in gvals:
                tl = cpool.tile([128, gv * 128], BF16, tag=f"iotar{gv}")
                nc.sync.dma_start(tl[:], iotarp[gv][:])
                iotar_sb[gv] = tl
            ident_sb = cpool.tile([128, 128], F32, tag="ident")
            nc.sync.dma_start(ident_sb[:], identp[:])
            idx_sb = cpool.tile([128, g_tot * 8], I16, tag="idx")
            nc.sync.dma_start(idx_sb[:], idxp[:])
            drel_sb = cpool.tile([128, g_tot], BF16, tag="drel")
            nc.sync.dma_start(drel_sb[:], drel[:])
            if with_b:
                b1_sb = cpool.tile([1, 128], F32, tag="b1")
                nc.sync.dma_start(b1_sb[:], b1p[:])
                b2_sb = cpool.tile([1, 128], F32, tag="b2")
                nc.sync.dma_start(b2_sb[:], b2p[:])
                ones_sb = cpool.tile([1, 128], F32, tag="ones")
                nc.gpsimd.memset(ones_sb[:], 1.0)
            lnla_sb = cpool.tile([128, 1], F32, tag="lnla")
            nc.gpsimd.memset(lnla_sb[:], float(np.log(LAM * ALPHA)))
            mla_sb = cpool.tile([128, 1], F32, tag="mla")
            nc.gpsimd.memset(mla_sb[:], float(-LAM * ALPHA))

            s1b = [None] * NBLK   # persistent (xc + noise1) wide blocks
            s2b = [None] * NBLK   # persistent (hc + noise2) wide blocks

            # ---- phase A: clip own x shard --------------------------------
            ssA = eps.tile([128, 64], F32, tag="ssA")
            xwb = []
            for b in range(NBLK):
                t0 = b * 4
                nb = min(4, NT - t0)
                xw = xin.tile([128, 512], F32, tag="xw")
                nc.sync.dma_start(_wide_sbuf(xw, nb), _wide_dram(xs, t0, nb))
                sqw = pa.tile([128, 512], F32, tag="sqw")
                for i in range(nb):
                    t = t0 + i
                    nc.scalar.activation(sqw[:, i * 128:(i + 1) * 128],
                                         xw[:, i * 128:(i + 1) * 128],
                                         Act.Square,
                                         accum_out=ssA[:, t:t + 1])
            ddA = eps.tile([128, 64], F32, tag="ddA")
            nc.vector.tensor_scalar_max(ddA[:, :NT], ssA[:, :NT], 1.0)
            rtA = eps.tile([128, 64], F32, tag="rtA")
            nc.scalar.activation(rtA[:, :NT], ddA[:, :NT], Act.Sqrt)
            scA = eps.tile([128, 64], F32, tag="scA")
            nc.vector.reciprocal(scA[:, :NT], rtA[:, :NT])
            stw = None
            ws = 0
            for b in range(NBLK):
                t0 = b * 4
                nb = min(4, NT - t0)
                nw = nrp.tile([128, 512], F32, tag="nwA")
                nc.sync.dma_start(_wide_sbuf(nw, nb), _wide_dram(n1s, t0, nb))
                xw2 = xin.tile([128, 512], F32, tag="xw")
                nc.sync.dma_start(_wide_sbuf(xw2, nb), _wide_dram(xs, t0, nb))
                s1 = selfp.tile([128, 512], BF16, tag=f"s1_{b}")
                if b % 2 == 0:
                    stw = stgb.tile([128, 1024], BF16, tag="stx")
                    ws = t0
                for i in range(nb):
                    t = t0 + i
                    cs = slice(i * 128, (i + 1) * 128)
                    nc.vector.scalar_tensor_tensor(
                        s1[:, cs], xw2[:, cs], scA[:, t:t + 1], nw[:, cs],
                        op0=mult, op1=add)
                    nc.scalar.activation(
                        stw[:, (t - ws) * 128:(t - ws + 1) * 128],
                        xw2[:, cs], Act.Copy, scale=scA[:, t:t + 1])
                s1b[b] = s1
                if b % 2 == 1 or b == NBLK - 1:
                    nbw = t0 + nb - ws
                    nc.sync.dma_start(_wide_dram(xtab, ws, nbw),
                                      _wide_sbuf(stw, nbw))

            # ---- edge phase ----------------------------------------------
            def edge_phase(tab, msgp, lname):
                ncalls = (g_tot + 7) // 8
                gts = []
                for k in range(ncalls):
                    ng = min(8, g_tot - k * 8)
                    gtile = gp.tile([128, 1024], BF16, tag=f"g{lname}")
                    nc.gpsimd.dma_gather(
                        gtile[:, :ng * 128].rearrange("p (g e) -> p g e", e=128),
                        tab[:, :],
                        idx_sb[:, k * 64:k * 64 + ng * 8],
                        ng * 128, ng * 128, 128)
                    gts.append(gtile)
                stw = pag = None
                ws = p4 = 0
                for (t0, R, Gv) in m["runs"]:
                    oh = ohp.tile([128, 2048], BF16, tag="oh")
                    _onehot_run(nc, oh, drel_sb, int(off[t0]), Gv, R,
                                iotar_sb[Gv])
                    for r in range(R):
                        t = t0 + r
                        ot = int(off[t])
                        if t % 16 == 0:
                            stw = stgp.tile([128, 2048], F16,
                                            tag=f"stp{lname}")
                            ws = t
                        if t % 4 == 0:
                            pag = psA.tile([128, 512], F32, tag="pag")
                            p4 = t
                        col = (t % 4) * 128
                        for j in range(Gv):
                            k, s = divmod(ot + j, 8)
                            nc.tensor.matmul(
                                pag[:, col:col + 128],
                                lhsT=_oh_col(oh, r, Gv, j),
                                rhs=gts[k][:, s * 128:(s + 1) * 128],
                                start=(j == 0), stop=(j == Gv - 1))
                        if t % 4 == 3 or t == GT - 1:
                            nbank = t - p4 + 1
                            dsl = stw[:, (p4 - ws) * 128:
                                      (p4 - ws + nbank) * 128]
                            nc.scalar.activation(dsl, pag[:, :nbank * 128],
                                                 Act.Copy)
                        if t % 16 == 15 or t == GT - 1:
                            nb = t - ws + 1
                            nc.sync.dma_start(_wide_dram(msgp, ws, nb),
                                              _wide_sbuf(stw, nb))

            # ---- epilogues -------------------------------------------------
            def epi_common(b, msgs, self_blk, wt_sb, b_sb, lname):
                """agg = self + msg; transpose; po = agg @ W.T. Returns po."""
                t0 = b * 4
                nb = min(4, NT - t0)
                w = nb * 128
                if b % 2 == 0:
                    nbm = min(8, NT - t0)
                    mwt = mrp.tile([128, 1024], F16, tag=f"mw{lname}")
                    nc.sync.dma_start(_wide_sbuf(mwt, nbm),
                                      _wide_dram(msgs, t0, nbm))
                    epi_common.mw = mwt
                mw = epi_common.mw
                aw = ep.tile([128, 512], F32, tag="aw")
                nc.gpsimd.tensor_tensor(
                    aw[:, :w], self_blk[:, :w],
                    mw[:, (b % 2) * 512:(b % 2) * 512 + w], op=add)
                pt = psT.tile([128, 512], F32, tag="pt")
                for i in range(nb):
                    cs = slice(i * 128, (i + 1) * 128)
                    nc.tensor.transpose(pt[:, cs], aw[:, cs], ident_sb[:])
                agT = ep.tile([128, 512], F32, tag="agT")
                if b % 2 == 0:
                    nc.vector.tensor_copy(agT[:, :w], pt[:, :w])
                else:
                    nc.scalar.activation(agT[:, :w], pt[:, :w], Act.Copy)
                po = psO.tile([128, 512], F32, tag="po")
                for i in range(nb):
                    cs = slice(i * 128, (i + 1) * 128)
                    if b_sb is not None:
                        nc.tensor.matmul(po[:, cs], lhsT=ones_sb[:], rhs=b_sb[:],
                                         start=True, stop=False)
                        nc.tensor.matmul(po[:, cs], lhsT=agT[:, cs], rhs=wt_sb[:],
                                         start=False, stop=True)
                    else:
                        nc.tensor.matmul(po[:, cs], lhsT=agT[:, cs], rhs=wt_sb[:],
                                         start=True, stop=True)
                return po, nb, w

            def epilogue1(msgs, noise, wt_sb, b_sb):
                ssL = eps.tile([128, 64], F32, tag="ssL")
                hhb = [None] * NBLK
                for b in range(NBLK):
                    po, nb, w = epi_common(b, msgs, s1b[b], wt_sb, b_sb, "1")
                    t0 = b * 4
                    m_ = ep.tile([128, 512], F32, tag="m_")
                    nc.scalar.activation(m_[:, :w], po[:, :w], Act.Relu)
                    tn = ep.tile([128, 512], F32, tag="tn")
                    nc.vector.tensor_tensor(tn[:, :w], po[:, :w], m_[:, :w],
                                            op=sub)
                    e_ = ep.tile([128, 512], F32, tag="e_")
                    nc.scalar.activation(e_[:, :w], tn[:, :w], Act.Exp,
                                         bias=lnla_sb[:])
                    hh = hhp.tile([128, 512], BF16, tag=f"hh{b}")
                    nc.vector.scalar_tensor_tensor(
                        hh[:, :w], m_[:, :w], float(LAM), e_[:, :w],
                        op0=mult, op1=add)
                    hhb[b] = hh
                    sqw = pa.tile([128, 512], F32, tag="sqe")
                    for i in range(nb):
                        t = t0 + i
                        cs = slice(i * 128, (i + 1) * 128)
                        nc.scalar.activation(sqw[:, cs], hh[:, cs], Act.Square,
                                             bias=mla_sb[:],
                                             accum_out=ssL[:, t:t + 1])
                # pass 2: clip scales, hc, s2, htab
                ddL = eps.tile([128, 64], F32, tag="ddL")
                nc.vector.tensor_scalar_max(ddL[:, :NT], ssL[:, :NT], 1.0)
                rtL = eps.tile([128, 64], F32, tag="rtL")
                nc.scalar.activation(rtL[:, :NT], ddL[:, :NT], Act.Sqrt)
                scL = eps.tile([128, 64], F32, tag="scL")
                nc.vector.reciprocal(scL[:, :NT], rtL[:, :NT])
                maL = eps.tile([128, 64], F32, tag="maL")
                nc.vector.tensor_scalar_mul(maL[:, :NT], scL[:, :NT],
                                            -LAM * ALPHA)
                stw = None
                ws = 0
                for b in range(NBLK):
                    t0 = b * 4
                    nb = min(4, NT - t0)
                    w = nb * 128
                    nw = nrp.tile([128, 512], F32, tag="nwB")
                    nc.sync.dma_start(_wide_sbuf(nw, nb),
                                      _wide_dram(noise, t0, nb))
                    hcw = ep.tile([128, 512], F32, tag="hcw")
                    for i in range(nb):
                        t = t0 + i
                        cs = slice(i * 128, (i + 1) * 128)
                        nc.vector.scalar_tensor_tensor(
                            hcw[:, cs], hhb[b][:, cs], scL[:, t:t + 1],
                            maL[:, t:t + 1].to_broadcast([128, 128]),
                            op0=mult, op1=add)
                    if b % 2 == 0:
                        stw = stgb.tile([128, 1024], BF16, tag="sth")
                        ws = t0
                    nc.scalar.activation(
                        stw[:, (t0 - ws) * 128:(t0 - ws) * 128 + w],
                        hcw[:, :w], Act.Copy)
                    s2 = selfp.tile([128, 512], BF16, tag=f"s2_{b}")
                    nc.gpsimd.tensor_tensor(s2[:, :w], hcw[:, :w], nw[:, :w],
                                            op=add)
                    s2b[b] = s2
                    if b % 2 == 1 or b == NBLK - 1:
                        nbw = t0 + nb - ws
                        nc.sync.dma_start(_wide_dram(htab, ws, nbw),
                                          _wide_sbuf(stw, nbw))

            def epilogue2(msgs, wt_sb, b_sb):
                stw = None
                ws = 0
                for b in range(NBLK):
                    po, nb, w = epi_common(b, msgs, s2b[b], wt_sb, b_sb, "2")
                    t0 = b * 4
                    if b % 2 == 0:
                        stw = stgp.tile([128, 1024], F32, tag="sto")
                        ws = t0
                    nc.scalar.activation(
                        stw[:, (t0 - ws) * 128:(t0 - ws) * 128 + w],
                        po[:, :w], Act.Copy)
                    if b % 2 == 1 or b == NBLK - 1:
                        nbw = t0 + nb - ws
                        nc.sync.dma_start(_wide_dram(outp, ws, nbw),
                                          _wide_sbuf(stw, nbw))

            # ---- layers ---------------------------------------------------
            edge_phase(xtab, msgp1, "a")
            nc.gpsimd.collective_compute(
                "ReduceScatter", add, ins=[msgp1[:, :]], outs=[msgs1[:, :]],
                replica_groups=rg)
            epilogue1(msgs1, n2s, w1t_sb, b1_sb if with_b else None)

            edge_phase(htab, msgp2, "b")
            nc.gpsimd.collective_compute(
                "ReduceScatter", add, ins=[msgp2[:, :]], outs=[msgs2[:, :]],
                replica_groups=rg)
            epilogue2(msgs2, w2t_sb, b2_sb if with_b else None)

    nc.compile()
    return nc


# ---------------------------------------------------------------------------
# Entry point
# ---------------------------------------------------------------------------

def _make_inmaps(inputs, meta, idx16, drel, with_b):
    S, SPAD, ncores = meta["S"], meta["SPAD"], meta["ncores"]
    n_nodes = meta["n_nodes"]
    x = np.ascontiguousarray(np.asarray(inputs["x"], np.float32))
    w1 = np.asarray(inputs["W1"], np.float32)
    w2 = np.asarray(inputs["W2"], np.float32)
    no1 = np.asarray(inputs["noise1"], np.float32)
    no2 = np.asarray(inputs["noise2"], np.float32)

    def shard(arr, c):
        lo = c * S
        hi = min(lo + S, n_nodes)
        out = np.zeros((SPAD, 128), np.float32)
        out[:hi - lo] = arr[lo:hi]
        return out

    import ml_dtypes
    ident = np.eye(128, dtype=np.float32)
    iotar = {}
    for _, _, gv in meta["runs"]:
        if gv not in iotar:
            iotar[gv] = np.ascontiguousarray(np.tile(
                (np.arange(gv * 128) // gv).astype(ml_dtypes.bfloat16),
                (128, 1)))
    in_maps = []
    for c in range(ncores):
        im = dict(
            xs=shard(x, c), n1s=shard(no1, c), n2s=shard(no2, c),
            w1t=np.ascontiguousarray(w1.T), w2t=np.ascontiguousarray(w2.T),
            idx=idx16[c], dstrel=drel[c], ident=ident,
            **{f"iotar{gv}": arr for gv, arr in iotar.items()},
        )
        if with_b:
            im["b1r"] = np.asarray(inputs["b1"], np.float32).reshape(1, 128)
            im["b2r"] = np.asarray(inputs["b2"], np.float32).reshape(1, 128)
        in_maps.append(im)
    return in_maps


def _run(inputs, ncores=NCORES, sim=False, trace=False):
    ei = np.asarray(inputs["edge_index"], np.int64)
    n_nodes = int(np.asarray(inputs["x"]).shape[0])
    meta, idx16, drel = _preprocess(ei[0], ei[1], n_nodes, ncores)
    with_b = bool(np.any(np.asarray(inputs["b1"])) or
                  np.any(np.asarray(inputs["b2"])))
    nc = _build_program(meta, with_b)
    in_maps = _make_inmaps(inputs, meta, idx16, drel, with_b)
    S = meta["S"]

    if sim:
        from concourse.bass_interp import MultiCoreSim
        msim = MultiCoreSim(nc, ncores, trace=trace)
        for c in range(ncores):
            for k, v in in_maps[c].items():
                msim.cores[c].tensor(k)[:] = v
        msim.simulate()
        results = [{"out": np.array(msim.cores[c].tensor("out"))}
                   for c in range(ncores)]
        res = msim
    else:
        res = run_bass_kernel_spmd(nc, in_maps, core_ids=list(range(ncores)),
                                   trace=trace)
        results = res.results

    parts = []
    for c in range(ncores):
        lo = c * S
        hi = min(lo + S, n_nodes)
        parts.append(results[c]["out"][:hi - lo])
    out = np.concatenate(parts, axis=0).astype(np.float32)
    return out, res


def kernel(**inputs) -> np.ndarray:
    out, _ = _run(inputs, ncores=NCORES, sim=False)
    return out


# revision 12
# speedup vs baseline: 1.9839x; 1.0262x over previous
"""Trainium2 Bass kernel for PrivateGraphSAGE (2-layer PrivSAGEConv).

Push-mode distribution (8 NeuronCores, SPMD):
  - Nodes (x, noise, output) sharded across cores (6250 rows each).
  - Edges partitioned by SOURCE owner: each core computes partial messages
    for ALL destinations using only its local clipped table (no AllGather).
  - Per 128-dst tile: dma_gather pulls source rows from the core's own bf16
    table; interleaved one-hots (bf16, built on DVE in one op per run of
    equal-G tiles) are the stationary matmul operands so the TensorEngine
    scatters segment-sums into wide [128,512] PSUM banks; one wide copy per
    bank stages partials which are written 8 tiles per DMA into a
    [50176,128] bf16 partial table.
  - A ReduceScatter(add) leaves each core its own destination shard of the
    summed messages (output N/8 -> ~55us vs ~250us for an AllGather).
  - Epilogues run in wide 4-tile blocks, two passes (SELU -> norms -> clip
    scale), with ACT pinned to the exp-family function set and the scalar
    chain on DVE to avoid activation-table reloads.
"""

import contextlib

import numpy as np

import concourse.bacc as bacc
import concourse.bass as bass
import concourse.mybir as mybir
import concourse.tile as tile
from concourse.bass_utils import run_bass_kernel_spmd

F32 = mybir.dt.float32
BF16 = mybir.dt.bfloat16
F16 = mybir.dt.bfloat16  # fp16 collectives unproven on NRT; bf16 is HW-validated
I16 = mybir.dt.int16

LAM = 1.0507009873554804934193349852946
ALPHA = 1.6732632423543772848170429916717

N_NODES = 50000
NCORES = 8


# ---------------------------------------------------------------------------
# Host-side preprocessing
# ---------------------------------------------------------------------------

def _preprocess(src, dst, n_nodes=N_NODES, ncores=NCORES):
    """Partition edges by source owner; bucket by global 128-dst tile; pad
    each bucket to G[tile]*128 edges with G uniform across cores (SPMD)."""
    S = -(-n_nodes // ncores)
    NT = -(-S // 128)
    SPAD = NT * 128
    GT = ncores * NT

    src = np.asarray(src, np.int64)
    dst = np.asarray(dst, np.int64)
    c = src // S
    lsrc = src - c * S
    cd = dst // S
    ld = dst - cd * S
    gt = cd * NT + ld // 128
    rel = ld % 128

    key = c * GT + gt
    counts = np.bincount(key, minlength=ncores * GT).reshape(ncores, GT)
    G = np.maximum(1, -(-counts.max(axis=0) // 128))
    off = np.concatenate([[0], np.cumsum(G)[:-1]]).astype(np.int64)
    g_tot = int(G.sum())
    e_pad = g_tot * 128

    order = np.argsort(key, kind="stable")
    key_s = key[order]
    lsrc_s = lsrc[order]
    rel_s = rel[order]
    run_start = np.concatenate(
        [[0], np.cumsum(np.bincount(key_s, minlength=ncores * GT))[:-1]])
    within = np.arange(len(key_s)) - run_start[key_s]
    slot = off[key_s % GT] * 128 + within
    cc = key_s // GT

    idxp = np.zeros((ncores, e_pad), np.int16)
    tagp = np.full((ncores, e_pad), -1.0, np.float32)
    idxp[cc, slot] = lsrc_s.astype(np.int16)
    tagp[cc, slot] = rel_s

    idx16 = idxp.reshape(ncores, e_pad // 16, 16).transpose(0, 2, 1)
    idx16 = np.ascontiguousarray(np.tile(idx16, (1, 8, 1)))

    import ml_dtypes
    drel = np.ascontiguousarray(
        tagp.reshape(ncores, g_tot, 128).transpose(0, 2, 1)
    ).astype(ml_dtypes.bfloat16)

    runs = []
    t = 0
    while t < GT:
        Gv = int(G[t])
        R = 1
        while (t + R < GT and int(G[t + R]) == Gv and (R + 1) * Gv <= 16
               and R < 8):
            R += 1
        runs.append((t, R, Gv))
        t += R

    meta = dict(n_nodes=n_nodes, ncores=ncores, S=S, NT=NT, SPAD=SPAD,
                GT=GT, G=G, off=off, g_tot=g_tot, runs=runs)
    return meta, idx16, drel


# ---------------------------------------------------------------------------
# Device program helpers
# ---------------------------------------------------------------------------

def _onehot_run(nc, oh, drel_sb, ot, Gv, R, iotar_sb):
    """One DVE op building interleaved one-hots for R consecutive tiles of
    G=Gv groups each: oh[e, r*Gv*128 + d*Gv + g] = (tag[e, ot+r*Gv+g] == d)."""
    a = oh[:]
    o4 = bass.AP(a.tensor, a.offset,
                 [list(a.ap[0]), [Gv * 128, R], [Gv, 128], [1, Gv]])
    d = drel_sb[:]
    d4 = bass.AP(d.tensor, d.offset + ot,
                 [list(d.ap[0]), [Gv, R], [0, 128], [1, Gv]])
    i = iotar_sb[:]
    i4 = bass.AP(i.tensor, i.offset,
                 [list(i.ap[0]), [0, R], [Gv, 128], [1, Gv]])
    nc.vector.tensor_tensor(o4, d4, i4, op=mybir.AluOpType.is_equal)


def _oh_col(oh, r, Gv, j):
    a = oh[:]
    return bass.AP(a.tensor, a.offset + r * Gv * 128 + j,
                   [list(a.ap[0]), [Gv, 128]])


def _wide_dram(t, r0, nrows):
    return t[r0 * 128:(r0 + nrows) * 128, :].rearrange("(g p) f -> p g f", p=128)


def _wide_sbuf(t, nrows, col0=0):
    return t[:, col0 * 128:(col0 + nrows) * 128].rearrange(
        "p (g f) -> p g f", f=128)


def _build_program(meta, with_b):
    m = meta
    S, NT, SPAD, GT = m["S"], m["NT"], m["SPAD"], m["GT"]
    G, off, g_tot = m["G"], m["off"], m["g_tot"]
    ncores = m["ncores"]
    NTAB = ncores * SPAD
    NBLK = (NT + 3) // 4
    rg = [list(range(ncores))]

    nc = bacc.Bacc(None, target_bir_lowering=False)

    xs = nc.declare_dram_parameter("xs", [SPAD, 128], F32, isOutput=False)
    n1s = nc.declare_dram_parameter("n1s", [SPAD, 128], F32, isOutput=False)
    n2s = nc.declare_dram_parameter("n2s", [SPAD, 128], F32, isOutput=False)
    w1t = nc.declare_dram_parameter("w1t", [128, 128], F32, isOutput=False)
    w2t = nc.declare_dram_parameter("w2t", [128, 128], F32, isOutput=False)
    idxp = nc.declare_dram_parameter("idx", [128, g_tot * 8], I16, isOutput=False)
    drel = nc.declare_dram_parameter("dstrel", [128, g_tot], BF16, isOutput=False)
    gvals = sorted({gv for _, _, gv in m["runs"]})
    iotarp = {gv: nc.declare_dram_parameter(f"iotar{gv}", [128, gv * 128],
                                            BF16, isOutput=False)
              for gv in gvals}
    identp = nc.declare_dram_parameter("ident", [128, 128], F32, isOutput=False)
    if with_b:
        b1p = nc.declare_dram_parameter("b1r", [1, 128], F32, isOutput=False)
        b2p = nc.declare_dram_parameter("b2r", [1, 128], F32, isOutput=False)
    outp = nc.declare_dram_parameter("out", [SPAD, 128], F32, isOutput=True)

    xtab = nc.dram_tensor("xtab", [SPAD, 128], BF16)
    htab = nc.dram_tensor("htab", [SPAD, 128], BF16)
    msgp1 = nc.dram_tensor("msgp1", [NTAB, 128], F16)
    msgp2 = nc.dram_tensor("msgp2", [NTAB, 128], F16)
    msgs1 = nc.dram_tensor("msgs1", [SPAD, 128], F16)
    msgs2 = nc.dram_tensor("msgs2", [SPAD, 128], F16)

    mult = mybir.AluOpType.mult
    add = mybir.AluOpType.add
    sub = mybir.AluOpType.subtract
    Act = mybir.ActivationFunctionType

    from concourse.library_config import mlp
    nc.gpsimd.load_library(mlp)

    with tile.TileContext(nc) as tc:
        with contextlib.ExitStack() as ctx:
            cpool = ctx.enter_context(tc.tile_pool(name="const", bufs=1))
            xin = ctx.enter_context(tc.tile_pool(name="xin", bufs=3))
            pa = ctx.enter_context(tc.tile_pool(name="pa", bufs=2))
            selfp = ctx.enter_context(tc.tile_pool(name="selfp", bufs=1))
            hhp = ctx.enter_context(tc.tile_pool(name="hhp", bufs=1))
            stgb = ctx.enter_context(tc.tile_pool(name="stgb", bufs=2))
            stgp = ctx.enter_context(tc.tile_pool(name="stgp", bufs=2))
            gp = ctx.enter_context(tc.tile_pool(name="gather", bufs=6))
            ohp = ctx.enter_context(tc.tile_pool(name="onehot", bufs=3))
            mrp = ctx.enter_context(tc.tile_pool(name="mread", bufs=2))
            nrp = ctx.enter_context(tc.tile_pool(name="nread", bufs=2))
            ep = ctx.enter_context(tc.tile_pool(name="epil", bufs=3))
            eps = ctx.enter_context(tc.tile_pool(name="epilsc", bufs=1))
            psA = ctx.enter_context(tc.tile_pool(name="psA", bufs=4, space="PSUM"))
            psT = ctx.enter_context(tc.tile_pool(name="psT", bufs=2, space="PSUM"))
            psO = ctx.enter_context(tc.tile_pool(name="psO", bufs=2, space="PSUM"))

            # ---- constants ------------------------------------------------
            w1t_sb = cpool.tile([128, 128], F32, tag="w1t")
            nc.sync.dma_start(w1t_sb[:], w1t[:])
            w2t_sb = cpool.tile([128, 128], F32, tag="w2t")
            nc.sync.dma_start(w2t_sb[:], w2t[:])
            iotar_sb = {}
            for gv in gvals:
                tl = cpool.tile([128, gv * 128], BF16, tag=f"iotar{gv}")
                nc.sync.dma_start(tl[:], iotarp[gv][:])
                iotar_sb[gv] = tl
            ident_sb = cpool.tile([128, 128], F32, tag="ident")
            nc.sync.dma_start(ident_sb[:], identp[:])
            idx_sb = cpool.tile([128, g_tot * 8], I16, tag="idx")
            nc.sync.dma_start(idx_sb[:], idxp[:])
            drel_sb = cpool.tile([128, g_tot], BF16, tag="drel")
            nc.sync.dma_start(drel_sb[:], drel[:])
            if with_b:
                b1_sb = cpool.tile([1, 128], F32, tag="b1")
                nc.sync.dma_start(b1_sb[:], b1p[:])
                b2_sb = cpool.tile([1, 128], F32, tag="b2")
                nc.sync.dma_start(b2_sb[:], b2p[:])
                ones_sb = cpool.tile([1, 128], F32, tag="ones")
                nc.gpsimd.memset(ones_sb[:], 1.0)
            lnla_sb = cpool.tile([128, 1], F32, tag="lnla")
            nc.gpsimd.memset(lnla_sb[:], float(np.log(LAM * ALPHA)))
            mla_sb = cpool.tile([128, 1], F32, tag="mla")
            nc.gpsimd.memset(mla_sb[:], float(-LAM * ALPHA))

            s1b = [None] * NBLK   # persistent (xc + noise1) wide blocks
            s2b = [None] * NBLK   # persistent (hc + noise2) wide blocks

            # ---- phase A: clip own x shard --------------------------------
            ssA = eps.tile([128, 64], F32, tag="ssA")
            xwb = []
            for b in range(NBLK):
                t0 = b * 4
                nb = min(4, NT - t0)
                xw = xin.tile([128, 512], F32, tag="xw")
                nc.sync.dma_start(_wide_sbuf(xw, nb), _wide_dram(xs, t0, nb))
                sqw = pa.tile([128, 512], F32, tag="sqw")
                for i in range(nb):
                    t = t0 + i
                    nc.scalar.activation(sqw[:, i * 128:(i + 1) * 128],
                                         xw[:, i * 128:(i + 1) * 128],
                                         Act.Square,
                                         accum_out=ssA[:, t:t + 1])
            ddA = eps.tile([128, 64], F32, tag="ddA")
            nc.vector.tensor_scalar_max(ddA[:, :NT], ssA[:, :NT], 1.0)
            rtA = eps.tile([128, 64], F32, tag="rtA")
            nc.scalar.activation(rtA[:, :NT], ddA[:, :NT], Act.Sqrt)
            scA = eps.tile([128, 64], F32, tag="scA")
            nc.vector.reciprocal(scA[:, :NT], rtA[:, :NT])
            stw = None
            ws = 0
            for b in range(NBLK):
                t0 = b * 4
                nb = min(4, NT - t0)
                nw = nrp.tile([128, 512], F32, tag="nwA")
                nc.sync.dma_start(_wide_sbuf(nw, nb), _wide_dram(n1s, t0, nb))
                xw2 = xin.tile([128, 512], F32, tag="xw")
                nc.sync.dma_start(_wide_sbuf(xw2, nb), _wide_dram(xs, t0, nb))
                s1 = selfp.tile([128, 512], BF16, tag=f"s1_{b}")
                if b % 2 == 0:
                    stw = stgb.tile([128, 1024], BF16, tag="stx")
                    ws = t0
                for i in range(nb):
                    t = t0 + i
                    cs = slice(i * 128, (i + 1) * 128)
                    nc.vector.scalar_tensor_tensor(
                        s1[:, cs], xw2[:, cs], scA[:, t:t + 1], nw[:, cs],
                        op0=mult, op1=add)
                    nc.scalar.activation(
                        stw[:, (t - ws) * 128:(t - ws + 1) * 128],
                        xw2[:, cs], Act.Copy, scale=scA[:, t:t + 1])
                s1b[b] = s1
                if b % 2 == 1 or b == NBLK - 1:
                    nbw = t0 + nb - ws
                    nc.sync.dma_start(_wide_dram(xtab, ws, nbw),
                                      _wide_sbuf(stw, nbw))

            # ---- edge phase ----------------------------------------------
            def edge_phase(tab, msgp, lname):
                ncalls = (g_tot + 7) // 8
                gts = []
                for k in range(ncalls):
                    ng = min(8, g_tot - k * 8)
                    gtile = gp.tile([128, 1024], BF16, tag=f"g{lname}")
                    nc.gpsimd.dma_gather(
                        gtile[:, :ng * 128].rearrange("p (g e) -> p g e", e=128),
                        tab[:, :],
                        idx_sb[:, k * 64:k * 64 + ng * 8],
                        ng * 128, ng * 128, 128)
                    gts.append(gtile)
                stw = pag = None
                ws = p4 = 0
                for (t0, R, Gv) in m["runs"]:
                    oh = ohp.tile([128, 2048], BF16, tag="oh")
                    _onehot_run(nc, oh, drel_sb, int(off[t0]), Gv, R,
                                iotar_sb[Gv])
                    for r in range(R):
                        t = t0 + r
                        ot = int(off[t])
                        if t % 16 == 0:
                            stw = stgp.tile([128, 2048], F16,
                                            tag=f"stp{lname}")
                            ws = t
                        if t % 4 == 0:
                            pag = psA.tile([128, 512], F32, tag="pag")
                            p4 = t
                        col = (t % 4) * 128
                        for j in range(Gv):
                            k, s = divmod(ot + j, 8)
                            nc.tensor.matmul(
                                pag[:, col:col + 128],
                                lhsT=_oh_col(oh, r, Gv, j),
                                rhs=gts[k][:, s * 128:(s + 1) * 128],
                                start=(j == 0), stop=(j == Gv - 1))
                        if t % 4 == 3 or t == GT - 1:
                            nbank = t - p4 + 1
                            dsl = stw[:, (p4 - ws) * 128:
                                      (p4 - ws + nbank) * 128]
                            nc.scalar.activation(dsl, pag[:, :nbank * 128],
                                                 Act.Copy)
                        if t % 16 == 15 or t == GT - 1:
                            nb = t - ws + 1
                            nc.sync.dma_start(_wide_dram(msgp, ws, nb),
                                              _wide_sbuf(stw, nb))

            # ---- epilogues -------------------------------------------------
            def epi_common(b, msgs, self_blk, wt_sb, b_sb, lname):
                """agg = self + msg; transpose; po = agg @ W.T. Returns po."""
                t0 = b * 4
                nb = min(4, NT - t0)
                w = nb * 128
                if b % 2 == 0:
                    nbm = min(8, NT - t0)
                    mwt = mrp.tile([128, 1024], F16, tag=f"mw{lname}")
                    nc.sync.dma_start(_wide_sbuf(mwt, nbm),
                                      _wide_dram(msgs, t0, nbm))
                    epi_common.mw = mwt
                mw = epi_common.mw
                aw = ep.tile([128, 512], F32, tag="aw")
                nc.gpsimd.tensor_tensor(
                    aw[:, :w], self_blk[:, :w],
                    mw[:, (b % 2) * 512:(b % 2) * 512 + w], op=add)
                pt = psT.tile([128, 512], F32, tag="pt")
                for i in range(nb):
                    cs = slice(i * 128, (i + 1) * 128)
                    nc.tensor.transpose(pt[:, cs], aw[:, cs], ident_sb[:])
                agT = ep.tile([128, 512], F32, tag="agT")
                if b % 2 == 0:
                    nc.vector.tensor_copy(agT[:, :w], pt[:, :w])
                else:
                    nc.scalar.activation(agT[:, :w], pt[:, :w], Act.Copy)
                po = psO.tile([128, 512], F32, tag="po")
                for i in range(nb):
                    cs = slice(i * 128, (i + 1) * 128)
                    if b_sb is not None:
                        nc.tensor.matmul(po[:, cs], lhsT=ones_sb[:], rhs=b_sb[:],
                                         start=True, stop=False)
                        nc.tensor.matmul(po[:, cs], lhsT=agT[:, cs], rhs=wt_sb[:],
                                         start=False, stop=True)
                    else:
                        nc.tensor.matmul(po[:, cs], lhsT=agT[:, cs], rhs=wt_sb[:],
                                         start=True, stop=True)
                return po, nb, w

            def epilogue1(msgs, noise, wt_sb, b_sb):
                ssL = eps.tile([128, 64], F32, tag="ssL")
                ddL = eps.tile([128, 64], F32, tag="ddL")
                rtL = eps.tile([128, 64], F32, tag="rtL")
                scL = eps.tile([128, 64], F32, tag="scL")
                maL = eps.tile([128, 64], F32, tag="maL")
                hhb = [None] * NBLK
                st = {"stw": None, "ws": 0}

                def pass1(b):
                    po, nb, w = epi_common(b, msgs, s1b[b], wt_sb, b_sb, "1")
                    t0 = b * 4
                    m_ = ep.tile([128, 512], F32, tag="m_")
                    nc.vector.tensor_scalar_max(m_[:, :w], po[:, :w], 0.0)
                    tn = ep.tile([128, 512], F32, tag="tn")
                    nc.vector.tensor_tensor(tn[:, :w], po[:, :w], m_[:, :w],
                                            op=sub)
                    e_ = ep.tile([128, 512], F32, tag="e_")
                    nc.scalar.activation(e_[:, :w], tn[:, :w], Act.Exp,
                                         bias=lnla_sb[:])
                    hh = hhp.tile([128, 512], BF16, tag=f"hh{b}")
                    nc.vector.scalar_tensor_tensor(
                        hh[:, :w], m_[:, :w], float(LAM), e_[:, :w],
                        op0=mult, op1=add)
                    hhb[b] = hh
                    sqw = pa.tile([128, 512], F32, tag="sqe")
                    for i in range(nb):
                        t = t0 + i
                        cs = slice(i * 128, (i + 1) * 128)
                        nc.scalar.activation(sqw[:, cs], hh[:, cs], Act.Square,
                                             bias=mla_sb[:],
                                             accum_out=ssL[:, t:t + 1])

                def scales(c0, c1):
                    nc.vector.tensor_scalar_max(ddL[:, c0:c1], ssL[:, c0:c1],
                                                1.0)
                    nc.scalar.activation(rtL[:, c0:c1], ddL[:, c0:c1],
                                         Act.Sqrt)
                    nc.vector.reciprocal(scL[:, c0:c1], rtL[:, c0:c1])
                    nc.vector.tensor_scalar_mul(maL[:, c0:c1], scL[:, c0:c1],
                                                -LAM * ALPHA)

                def pass2(b):
                    t0 = b * 4
                    nb = min(4, NT - t0)
                    w = nb * 128
                    nw = nrp.tile([128, 512], F32, tag="nwB")
                    nc.sync.dma_start(_wide_sbuf(nw, nb),
                                      _wide_dram(noise, t0, nb))
                    hcw = ep.tile([128, 512], F32, tag="hcw")
                    for i in range(nb):
                        t = t0 + i
                        cs = slice(i * 128, (i + 1) * 128)
                        nc.vector.scalar_tensor_tensor(
                            hcw[:, cs], hhb[b][:, cs], scL[:, t:t + 1],
                            maL[:, t:t + 1].to_broadcast([128, 128]),
                            op0=mult, op1=add)
                    if b % 2 == 0:
                        sthw = stgb.tile([128, 1024], BF16, tag="sth")
                        st["stw"] = sthw
                        st["ws"] = t0
                    nc.scalar.activation(
                        st["stw"][:, (t0 - st["ws"]) * 128:
                                  (t0 - st["ws"]) * 128 + w],
                        hcw[:, :w], Act.Copy)
                    s2 = selfp.tile([128, 512], BF16, tag=f"s2_{b}")
                    nc.gpsimd.tensor_tensor(s2[:, :w], hcw[:, :w], nw[:, :w],
                                            op=add)
                    s2b[b] = s2
                    if b % 2 == 1 or b == NBLK - 1:
                        nbw = t0 + nb - st["ws"]
                        nc.sync.dma_start(_wide_dram(htab, st["ws"], nbw),
                                          _wide_sbuf(st["stw"], nbw))

                # half-pipelined: pass2 of the first 7 blocks (tiles 0..27)
                # interleaves with pass1 of the last 6 blocks.
                for b in range(7):
                    pass1(b)
                scales(0, 28)
                for k in range(6):
                    pass1(7 + k)
                    pass2(k)
                pass2(6)
                scales(28, NT)
                for b in range(7, NBLK):
                    pass2(b)

            def epilogue2(msgs, wt_sb, b_sb):
                stw = None
                ws = 0
                for b in range(NBLK):
                    po, nb, w = epi_common(b, msgs, s2b[b], wt_sb, b_sb, "2")
                    t0 = b * 4
                    if b % 2 == 0:
                        stw = stgp.tile([128, 1024], F32, tag="sto")
                        ws = t0
                    nc.scalar.activation(
                        stw[:, (t0 - ws) * 128:(t0 - ws) * 128 + w],
                        po[:, :w], Act.Copy)
                    if b % 2 == 1 or b == NBLK - 1:
                        nbw = t0 + nb - ws
                        nc.sync.dma_start(_wide_dram(outp, ws, nbw),
                                          _wide_sbuf(stw, nbw))

            # ---- layers ---------------------------------------------------
            edge_phase(xtab, msgp1, "a")
            nc.gpsimd.collective_compute(
                "ReduceScatter", add, ins=[msgp1[:, :]], outs=[msgs1[:, :]],
                replica_groups=rg)
            epilogue1(msgs1, n2s, w1t_sb, b1_sb if with_b else None)

            edge_phase(htab, msgp2, "b")
            nc.gpsimd.collective_compute(
                "ReduceScatter", add, ins=[msgp2[:, :]], outs=[msgs2[:, :]],
                replica_groups=rg)
            epilogue2(msgs2, w2t_sb, b2_sb if with_b else None)

    nc.compile()
    return nc


# ---------------------------------------------------------------------------
# Entry point
# ---------------------------------------------------------------------------

def _make_inmaps(inputs, meta, idx16, drel, with_b):
    S, SPAD, ncores = meta["S"], meta["SPAD"], meta["ncores"]
    n_nodes = meta["n_nodes"]
    x = np.ascontiguousarray(np.asarray(inputs["x"], np.float32))
    w1 = np.asarray(inputs["W1"], np.float32)
    w2 = np.asarray(inputs["W2"], np.float32)
    no1 = np.asarray(inputs["noise1"], np.float32)
    no2 = np.asarray(inputs["noise2"], np.float32)

    def shard(arr, c):
        lo = c * S
        hi = min(lo + S, n_nodes)
        out = np.zeros((SPAD, 128), np.float32)
        out[:hi - lo] = arr[lo:hi]
        return out

    import ml_dtypes
    ident = np.eye(128, dtype=np.float32)
    iotar = {}
    for _, _, gv in meta["runs"]:
        if gv not in iotar:
            iotar[gv] = np.ascontiguousarray(np.tile(
                (np.arange(gv * 128) // gv).astype(ml_dtypes.bfloat16),
                (128, 1)))
    in_maps = []
    for c in range(ncores):
        im = dict(
            xs=shard(x, c), n1s=shard(no1, c), n2s=shard(no2, c),
            w1t=np.ascontiguousarray(w1.T), w2t=np.ascontiguousarray(w2.T),
            idx=idx16[c], dstrel=drel[c], ident=ident,
            **{f"iotar{gv}": arr for gv, arr in iotar.items()},
        )
        if with_b:
            im["b1r"] = np.asarray(inputs["b1"], np.float32).reshape(1, 128)
            im["b2r"] = np.asarray(inputs["b2"], np.float32).reshape(1, 128)
        in_maps.append(im)
    return in_maps


def _run(inputs, ncores=NCORES, sim=False, trace=False):
    ei = np.asarray(inputs["edge_index"], np.int64)
    n_nodes = int(np.asarray(inputs["x"]).shape[0])
    meta, idx16, drel = _preprocess(ei[0], ei[1], n_nodes, ncores)
    with_b = bool(np.any(np.asarray(inputs["b1"])) or
                  np.any(np.asarray(inputs["b2"])))
    nc = _build_program(meta, with_b)
    in_maps = _make_inmaps(inputs, meta, idx16, drel, with_b)
    S = meta["S"]

    if sim:
        from concourse.bass_interp import MultiCoreSim
        msim = MultiCoreSim(nc, ncores, trace=trace)
        for c in range(ncores):
            for k, v in in_maps[c].items():
                msim.cores[c].tensor(k)[:] = v
        msim.simulate()
        results = [{"out": np.array(msim.cores[c].tensor("out"))}
                   for c in range(ncores)]
        res = msim
    else:
        res = run_bass_kernel_spmd(nc, in_maps, core_ids=list(range(ncores)),
                                   trace=trace)
        results = res.results

    parts = []
    for c in range(ncores):
        lo = c * S
        hi = min(lo + S, n_nodes)
        parts.append(results[c]["out"][:hi - lo])
    out = np.concatenate(parts, axis=0).astype(np.float32)
    return out, res


def kernel(**inputs) -> np.ndarray:
    out, _ = _run(inputs, ncores=NCORES, sim=False)
    return out


# revision 13
# speedup vs baseline: 1.9960x; 1.0061x over previous
"""Trainium2 Bass kernel for PrivateGraphSAGE (2-layer PrivSAGEConv).

Push-mode distribution (8 NeuronCores, SPMD):
  - Nodes (x, noise, output) sharded across cores (6250 rows each).
  - Edges partitioned by SOURCE owner: each core computes partial messages
    for ALL destinations using only its local clipped table (no AllGather).
  - Per 128-dst tile: dma_gather pulls source rows from the core's own bf16
    table; interleaved one-hots (bf16, built on DVE in one op per run of
    equal-G tiles) are the stationary matmul operands so the TensorEngine
    scatters segment-sums into wide [128,512] PSUM banks; one wide copy per
    bank stages partials which are written 8 tiles per DMA into a
    [50176,128] bf16 partial table.
  - A ReduceScatter(add) leaves each core its own destination shard of the
    summed messages (output N/8 -> ~55us vs ~250us for an AllGather).
  - Epilogues run in wide 4-tile blocks, two passes (SELU -> norms -> clip
    scale), with ACT pinned to the exp-family function set and the scalar
    chain on DVE to avoid activation-table reloads.
"""

import contextlib

import numpy as np

import concourse.bacc as bacc
import concourse.bass as bass
import concourse.mybir as mybir
import concourse.tile as tile
from concourse.bass_utils import run_bass_kernel_spmd

F32 = mybir.dt.float32
BF16 = mybir.dt.bfloat16
F16 = mybir.dt.bfloat16  # fp16 collectives unproven on NRT; bf16 is HW-validated
I16 = mybir.dt.int16

LAM = 1.0507009873554804934193349852946
ALPHA = 1.6732632423543772848170429916717

N_NODES = 50000
NCORES = 8


# ---------------------------------------------------------------------------
# Host-side preprocessing
# ---------------------------------------------------------------------------

def _preprocess(src, dst, n_nodes=N_NODES, ncores=NCORES):
    """Partition edges by source owner; bucket by global 128-dst tile; pad
    each bucket to G[tile]*128 edges with G uniform across cores (SPMD)."""
    S = -(-n_nodes // ncores)
    NT = -(-S // 128)
    SPAD = NT * 128
    GT = ncores * NT

    src = np.asarray(src, np.int64)
    dst = np.asarray(dst, np.int64)
    c = src // S
    lsrc = src - c * S
    cd = dst // S
    ld = dst - cd * S
    gt = cd * NT + ld // 128
    rel = ld % 128

    key = c * GT + gt
    counts = np.bincount(key, minlength=ncores * GT).reshape(ncores, GT)
    G = np.maximum(1, -(-counts.max(axis=0) // 128))
    off = np.concatenate([[0], np.cumsum(G)[:-1]]).astype(np.int64)
    g_tot = int(G.sum())
    e_pad = g_tot * 128

    order = np.argsort(key, kind="stable")
    key_s = key[order]
    lsrc_s = lsrc[order]
    rel_s = rel[order]
    run_start = np.concatenate(
        [[0], np.cumsum(np.bincount(key_s, minlength=ncores * GT))[:-1]])
    within = np.arange(len(key_s)) - run_start[key_s]
    slot = off[key_s % GT] * 128 + within
    cc = key_s // GT

    idxp = np.zeros((ncores, e_pad), np.int16)
    tagp = np.full((ncores, e_pad), -1.0, np.float32)
    idxp[cc, slot] = lsrc_s.astype(np.int16)
    tagp[cc, slot] = rel_s

    idx16 = idxp.reshape(ncores, e_pad // 16, 16).transpose(0, 2, 1)
    idx16 = np.ascontiguousarray(np.tile(idx16, (1, 8, 1)))

    import ml_dtypes
    drel = np.ascontiguousarray(
        tagp.reshape(ncores, g_tot, 128).transpose(0, 2, 1)
    ).astype(ml_dtypes.bfloat16)

    runs = []
    t = 0
    while t < GT:
        Gv = int(G[t])
        R = 1
        while (t + R < GT and int(G[t + R]) == Gv and (R + 1) * Gv <= 16
               and R < 8):
            R += 1
        runs.append((t, R, Gv))
        t += R

    meta = dict(n_nodes=n_nodes, ncores=ncores, S=S, NT=NT, SPAD=SPAD,
                GT=GT, G=G, off=off, g_tot=g_tot, runs=runs)
    return meta, idx16, drel


# ---------------------------------------------------------------------------
# Device program helpers
# ---------------------------------------------------------------------------

def _onehot_run(nc, oh, drel_sb, ot, Gv, R, iotar_sb):
    """One DVE op building interleaved one-hots for R consecutive tiles of
    G=Gv groups each: oh[e, r*Gv*128 + d*Gv + g] = (tag[e, ot+r*Gv+g] == d)."""
    a = oh[:]
    o4 = bass.AP(a.tensor, a.offset,
                 [list(a.ap[0]), [Gv * 128, R], [Gv, 128], [1, Gv]])
    d = drel_sb[:]
    d4 = bass.AP(d.tensor, d.offset + ot,
                 [list(d.ap[0]), [Gv, R], [0, 128], [1, Gv]])
    i = iotar_sb[:]
    i4 = bass.AP(i.tensor, i.offset,
                 [list(i.ap[0]), [0, R], [Gv, 128], [1, Gv]])
    nc.vector.tensor_tensor(o4, d4, i4, op=mybir.AluOpType.is_equal)


def _oh_col(oh, r, Gv, j):
    a = oh[:]
    return bass.AP(a.tensor, a.offset + r * Gv * 128 + j,
                   [list(a.ap[0]), [Gv, 128]])


def _wide_dram(t, r0, nrows):
    return t[r0 * 128:(r0 + nrows) * 128, :].rearrange("(g p) f -> p g f", p=128)


def _wide_sbuf(t, nrows, col0=0):
    return t[:, col0 * 128:(col0 + nrows) * 128].rearrange(
        "p (g f) -> p g f", f=128)


def _build_program(meta, with_b):
    m = meta
    S, NT, SPAD, GT = m["S"], m["NT"], m["SPAD"], m["GT"]
    G, off, g_tot = m["G"], m["off"], m["g_tot"]
    ncores = m["ncores"]
    NTAB = ncores * SPAD
    NBLK = (NT + 3) // 4
    rg = [list(range(ncores))]

    nc = bacc.Bacc(None, target_bir_lowering=False)

    xs = nc.declare_dram_parameter("xs", [SPAD, 128], F32, isOutput=False)
    n1s = nc.declare_dram_parameter("n1s", [SPAD, 128], F32, isOutput=False)
    n2s = nc.declare_dram_parameter("n2s", [SPAD, 128], F32, isOutput=False)
    w1t = nc.declare_dram_parameter("w1t", [128, 128], F32, isOutput=False)
    w2t = nc.declare_dram_parameter("w2t", [128, 128], F32, isOutput=False)
    idxp = nc.declare_dram_parameter("idx", [128, g_tot * 8], I16, isOutput=False)
    drel = nc.declare_dram_parameter("dstrel", [128, g_tot], BF16, isOutput=False)
    gvals = sorted({gv for _, _, gv in m["runs"]})
    iotarp = {gv: nc.declare_dram_parameter(f"iotar{gv}", [128, gv * 128],
                                            BF16, isOutput=False)
              for gv in gvals}
    identp = nc.declare_dram_parameter("ident", [128, 128], F32, isOutput=False)
    if with_b:
        b1p = nc.declare_dram_parameter("b1r", [1, 128], F32, isOutput=False)
        b2p = nc.declare_dram_parameter("b2r", [1, 128], F32, isOutput=False)
    outp = nc.declare_dram_parameter("out", [SPAD, 128], F32, isOutput=True)

    xtab = nc.dram_tensor("xtab", [SPAD, 128], BF16)
    htab = nc.dram_tensor("htab", [SPAD, 128], BF16)
    msgp1 = nc.dram_tensor("msgp1", [NTAB, 128], F16)
    msgp2 = nc.dram_tensor("msgp2", [NTAB, 128], F16)
    msgs1 = nc.dram_tensor("msgs1", [SPAD, 128], F16)
    msgs2 = nc.dram_tensor("msgs2", [SPAD, 128], F16)

    mult = mybir.AluOpType.mult
    add = mybir.AluOpType.add
    sub = mybir.AluOpType.subtract
    Act = mybir.ActivationFunctionType

    from concourse.library_config import mlp
    nc.gpsimd.load_library(mlp)

    with tile.TileContext(nc) as tc:
        with contextlib.ExitStack() as ctx:
            cpool = ctx.enter_context(tc.tile_pool(name="const", bufs=1))
            xin = ctx.enter_context(tc.tile_pool(name="xin", bufs=3))
            pa = ctx.enter_context(tc.tile_pool(name="pa", bufs=2))
            selfp = ctx.enter_context(tc.tile_pool(name="selfp", bufs=1))
            hhp = ctx.enter_context(tc.tile_pool(name="hhp", bufs=1))
            stgb = ctx.enter_context(tc.tile_pool(name="stgb", bufs=2))
            stgp = ctx.enter_context(tc.tile_pool(name="stgp", bufs=2))
            gp = ctx.enter_context(tc.tile_pool(name="gather", bufs=6))
            ohp = ctx.enter_context(tc.tile_pool(name="onehot", bufs=3))
            mrp = ctx.enter_context(tc.tile_pool(name="mread", bufs=3))
            nrp = ctx.enter_context(tc.tile_pool(name="nread", bufs=2))
            ep = ctx.enter_context(tc.tile_pool(name="epil", bufs=4))
            eps = ctx.enter_context(tc.tile_pool(name="epilsc", bufs=1))
            psA = ctx.enter_context(tc.tile_pool(name="psA", bufs=4, space="PSUM"))
            psT = ctx.enter_context(tc.tile_pool(name="psT", bufs=2, space="PSUM"))
            psO = ctx.enter_context(tc.tile_pool(name="psO", bufs=2, space="PSUM"))

            # ---- constants ------------------------------------------------
            w1t_sb = cpool.tile([128, 128], F32, tag="w1t")
            nc.sync.dma_start(w1t_sb[:], w1t[:])
            w2t_sb = cpool.tile([128, 128], F32, tag="w2t")
            nc.sync.dma_start(w2t_sb[:], w2t[:])
            iotar_sb = {}
            for gv in gvals:
                tl = cpool.tile([128, gv * 128], BF16, tag=f"iotar{gv}")
                nc.sync.dma_start(tl[:], iotarp[gv][:])
                iotar_sb[gv] = tl
            ident_sb = cpool.tile([128, 128], F32, tag="ident")
            nc.sync.dma_start(ident_sb[:], identp[:])
            idx_sb = cpool.tile([128, g_tot * 8], I16, tag="idx")
            nc.sync.dma_start(idx_sb[:], idxp[:])
            drel_sb = cpool.tile([128, g_tot], BF16, tag="drel")
            nc.sync.dma_start(drel_sb[:], drel[:])
            if with_b:
                b1_sb = cpool.tile([1, 128], F32, tag="b1")
                nc.sync.dma_start(b1_sb[:], b1p[:])
                b2_sb = cpool.tile([1, 128], F32, tag="b2")
                nc.sync.dma_start(b2_sb[:], b2p[:])
                ones_sb = cpool.tile([1, 128], F32, tag="ones")
                nc.gpsimd.memset(ones_sb[:], 1.0)
            lnla_sb = cpool.tile([128, 1], F32, tag="lnla")
            nc.gpsimd.memset(lnla_sb[:], float(np.log(LAM * ALPHA)))
            mla_sb = cpool.tile([128, 1], F32, tag="mla")
            nc.gpsimd.memset(mla_sb[:], float(-LAM * ALPHA))

            s1b = [None] * NBLK   # persistent (xc + noise1) wide blocks
            s2b = [None] * NBLK   # persistent (hc + noise2) wide blocks

            # ---- phase A: clip own x shard --------------------------------
            ssA = eps.tile([128, 64], F32, tag="ssA")
            ddA = eps.tile([128, 64], F32, tag="ddA")
            rtA = eps.tile([128, 64], F32, tag="rtA")
            scA = eps.tile([128, 64], F32, tag="scA")
            stA = {"stw": None, "ws": 0}

            def a_pass1(b):
                t0 = b * 4
                nb = min(4, NT - t0)
                xw = xin.tile([128, 512], F32, tag="xw")
                nc.sync.dma_start(_wide_sbuf(xw, nb), _wide_dram(xs, t0, nb))
                sqw = pa.tile([128, 512], F32, tag="sqw")
                for i in range(nb):
                    t = t0 + i
                    nc.scalar.activation(sqw[:, i * 128:(i + 1) * 128],
                                         xw[:, i * 128:(i + 1) * 128],
                                         Act.Square,
                                         accum_out=ssA[:, t:t + 1])

            def a_scales(c0, c1):
                nc.vector.tensor_scalar_max(ddA[:, c0:c1], ssA[:, c0:c1], 1.0)
                nc.scalar.activation(rtA[:, c0:c1], ddA[:, c0:c1], Act.Sqrt)
                nc.vector.reciprocal(scA[:, c0:c1], rtA[:, c0:c1])

            def a_pass2(b):
                t0 = b * 4
                nb = min(4, NT - t0)
                nw = nrp.tile([128, 512], F32, tag="nwA")
                nc.sync.dma_start(_wide_sbuf(nw, nb), _wide_dram(n1s, t0, nb))
                xw2 = xin.tile([128, 512], F32, tag="xw")
                nc.sync.dma_start(_wide_sbuf(xw2, nb), _wide_dram(xs, t0, nb))
                s1 = selfp.tile([128, 512], BF16, tag=f"s1_{b}")
                if b % 2 == 0:
                    stxw = stgb.tile([128, 1024], BF16, tag="stx")
                    stA["stw"] = stxw
                    stA["ws"] = t0
                for i in range(nb):
                    t = t0 + i
                    cs = slice(i * 128, (i + 1) * 128)
                    nc.vector.scalar_tensor_tensor(
                        s1[:, cs], xw2[:, cs], scA[:, t:t + 1], nw[:, cs],
                        op0=mult, op1=add)
                    nc.scalar.activation(
                        stA["stw"][:, (t - stA["ws"]) * 128:
                                   (t - stA["ws"] + 1) * 128],
                        xw2[:, cs], Act.Copy, scale=scA[:, t:t + 1])
                s1b[b] = s1
                if b % 2 == 1 or b == NBLK - 1:
                    nbw = t0 + nb - stA["ws"]
                    nc.sync.dma_start(_wide_dram(xtab, stA["ws"], nbw),
                                      _wide_sbuf(stA["stw"], nbw))

            for b in range(7):
                a_pass1(b)
            a_scales(0, 28)
            for k in range(6):
                a_pass1(7 + k)
                a_pass2(k)
            a_pass2(6)
            a_scales(28, NT)
            for b in range(7, NBLK):
                a_pass2(b)

            # ---- edge phase ----------------------------------------------
            def edge_phase(tab, msgp, lname):
                ncalls = (g_tot + 7) // 8
                gts = []
                for k in range(ncalls):
                    ng = min(8, g_tot - k * 8)
                    gtile = gp.tile([128, 1024], BF16, tag=f"g{lname}")
                    nc.gpsimd.dma_gather(
                        gtile[:, :ng * 128].rearrange("p (g e) -> p g e", e=128),
                        tab[:, :],
                        idx_sb[:, k * 64:k * 64 + ng * 8],
                        ng * 128, ng * 128, 128)
                    gts.append(gtile)
                stw = pag = None
                ws = p4 = 0
                for (t0, R, Gv) in m["runs"]:
                    oh = ohp.tile([128, 2048], BF16, tag="oh")
                    _onehot_run(nc, oh, drel_sb, int(off[t0]), Gv, R,
                                iotar_sb[Gv])
                    for r in range(R):
                        t = t0 + r
                        ot = int(off[t])
                        if t % 16 == 0:
                            stw = stgp.tile([128, 2048], F16,
                                            tag=f"stp{lname}")
                            ws = t
                        if t % 4 == 0:
                            pag = psA.tile([128, 512], F32, tag="pag")
                            p4 = t
                        col = (t % 4) * 128
                        for j in range(Gv):
                            k, s = divmod(ot + j, 8)
                            nc.tensor.matmul(
                                pag[:, col:col + 128],
                                lhsT=_oh_col(oh, r, Gv, j),
                                rhs=gts[k][:, s * 128:(s + 1) * 128],
                                start=(j == 0), stop=(j == Gv - 1))
                        if t % 4 == 3 or t == GT - 1:
                            nbank = t - p4 + 1
                            dsl = stw[:, (p4 - ws) * 128:
                                      (p4 - ws + nbank) * 128]
                            nc.scalar.activation(dsl, pag[:, :nbank * 128],
                                                 Act.Copy)
                        if t % 16 == 15 or t == GT - 1:
                            nb = t - ws + 1
                            nc.sync.dma_start(_wide_dram(msgp, ws, nb),
                                              _wide_sbuf(stw, nb))

            # ---- epilogues -------------------------------------------------
            def epi_common(b, msgs, self_blk, wt_sb, b_sb, lname):
                """agg = self + msg; transpose; po = agg @ W.T. Returns po."""
                t0 = b * 4
                nb = min(4, NT - t0)
                w = nb * 128
                if b % 2 == 0:
                    nbm = min(8, NT - t0)
                    mwt = mrp.tile([128, 1024], F16, tag=f"mw{lname}")
                    nc.sync.dma_start(_wide_sbuf(mwt, nbm),
                                      _wide_dram(msgs, t0, nbm))
                    epi_common.mw = mwt
                mw = epi_common.mw
                aw = ep.tile([128, 512], F32, tag="aw")
                nc.gpsimd.tensor_tensor(
                    aw[:, :w], self_blk[:, :w],
                    mw[:, (b % 2) * 512:(b % 2) * 512 + w], op=add)
                pt = psT.tile([128, 512], F32, tag="pt")
                for i in range(nb):
                    cs = slice(i * 128, (i + 1) * 128)
                    nc.tensor.transpose(pt[:, cs], aw[:, cs], ident_sb[:])
                agT = ep.tile([128, 512], F32, tag="agT")
                if b % 2 == 0:
                    nc.vector.tensor_copy(agT[:, :w], pt[:, :w])
                else:
                    nc.scalar.activation(agT[:, :w], pt[:, :w], Act.Copy)
                po = psO.tile([128, 512], F32, tag="po")
                for i in range(nb):
                    cs = slice(i * 128, (i + 1) * 128)
                    if b_sb is not None:
                        nc.tensor.matmul(po[:, cs], lhsT=ones_sb[:], rhs=b_sb[:],
                                         start=True, stop=False)
                        nc.tensor.matmul(po[:, cs], lhsT=agT[:, cs], rhs=wt_sb[:],
                                         start=False, stop=True)
                    else:
                        nc.tensor.matmul(po[:, cs], lhsT=agT[:, cs], rhs=wt_sb[:],
                                         start=True, stop=True)
                return po, nb, w

            def epilogue1(msgs, noise, wt_sb, b_sb):
                ssL = eps.tile([128, 64], F32, tag="ssL")
                ddL = eps.tile([128, 64], F32, tag="ddL")
                rtL = eps.tile([128, 64], F32, tag="rtL")
                scL = eps.tile([128, 64], F32, tag="scL")
                maL = eps.tile([128, 64], F32, tag="maL")
                hhb = [None] * NBLK
                st = {"stw": None, "ws": 0}

                def pass1(b):
                    po, nb, w = epi_common(b, msgs, s1b[b], wt_sb, b_sb, "1")
                    t0 = b * 4
                    m_ = ep.tile([128, 512], F32, tag="m_")
                    nc.vector.tensor_scalar_max(m_[:, :w], po[:, :w], 0.0)
                    tn = ep.tile([128, 512], F32, tag="tn")
                    nc.vector.tensor_tensor(tn[:, :w], po[:, :w], m_[:, :w],
                                            op=sub)
                    e_ = ep.tile([128, 512], F32, tag="e_")
                    nc.scalar.activation(e_[:, :w], tn[:, :w], Act.Exp,
                                         bias=lnla_sb[:])
                    hh = hhp.tile([128, 512], BF16, tag=f"hh{b}")
                    nc.vector.scalar_tensor_tensor(
                        hh[:, :w], m_[:, :w], float(LAM), e_[:, :w],
                        op0=mult, op1=add)
                    hhb[b] = hh
                    sqw = pa.tile([128, 512], F32, tag="sqe")
                    for i in range(nb):
                        t = t0 + i
                        cs = slice(i * 128, (i + 1) * 128)
                        nc.scalar.activation(sqw[:, cs], hh[:, cs], Act.Square,
                                             bias=mla_sb[:],
                                             accum_out=ssL[:, t:t + 1])

                def scales(c0, c1):
                    nc.vector.tensor_scalar_max(ddL[:, c0:c1], ssL[:, c0:c1],
                                                1.0)
                    nc.scalar.activation(rtL[:, c0:c1], ddL[:, c0:c1],
                                         Act.Sqrt)
                    nc.vector.reciprocal(scL[:, c0:c1], rtL[:, c0:c1])
                    nc.vector.tensor_scalar_mul(maL[:, c0:c1], scL[:, c0:c1],
                                                -LAM * ALPHA)

                def pass2(b):
                    t0 = b * 4
                    nb = min(4, NT - t0)
                    w = nb * 128
                    nw = nrp.tile([128, 512], F32, tag="nwB")
                    nc.sync.dma_start(_wide_sbuf(nw, nb),
                                      _wide_dram(noise, t0, nb))
                    hcw = ep.tile([128, 512], F32, tag="hcw")
                    for i in range(nb):
                        t = t0 + i
                        cs = slice(i * 128, (i + 1) * 128)
                        nc.vector.scalar_tensor_tensor(
                            hcw[:, cs], hhb[b][:, cs], scL[:, t:t + 1],
                            maL[:, t:t + 1].to_broadcast([128, 128]),
                            op0=mult, op1=add)
                    if b % 2 == 0:
                        sthw = stgb.tile([128, 1024], BF16, tag="sth")
                        st["stw"] = sthw
                        st["ws"] = t0
                    nc.scalar.activation(
                        st["stw"][:, (t0 - st["ws"]) * 128:
                                  (t0 - st["ws"]) * 128 + w],
                        hcw[:, :w], Act.Copy)
                    s2 = selfp.tile([128, 512], BF16, tag=f"s2_{b}")
                    nc.gpsimd.tensor_tensor(s2[:, :w], hcw[:, :w], nw[:, :w],
                                            op=add)
                    s2b[b] = s2
                    if b % 2 == 1 or b == NBLK - 1:
                        nbw = t0 + nb - st["ws"]
                        nc.sync.dma_start(_wide_dram(htab, st["ws"], nbw),
                                          _wide_sbuf(st["stw"], nbw))

                # half-pipelined: pass2 of the first 7 blocks (tiles 0..27)
                # interleaves with pass1 of the last 6 blocks.
                for b in range(7):
                    pass1(b)
                scales(0, 28)
                for k in range(6):
                    pass1(7 + k)
                    pass2(k)
                pass2(6)
                scales(28, NT)
                for b in range(7, NBLK):
                    pass2(b)

            def epilogue2(msgs, wt_sb, b_sb):
                stw = None
                ws = 0
                for b in range(NBLK):
                    po, nb, w = epi_common(b, msgs, s2b[b], wt_sb, b_sb, "2")
                    t0 = b * 4
                    if b % 2 == 0:
                        stw = stgp.tile([128, 1024], F32, tag="sto")
                        ws = t0
                    nc.scalar.activation(
                        stw[:, (t0 - ws) * 128:(t0 - ws) * 128 + w],
                        po[:, :w], Act.Copy)
                    if b % 2 == 1 or b == NBLK - 1:
                        nbw = t0 + nb - ws
                        nc.sync.dma_start(_wide_dram(outp, ws, nbw),
                                          _wide_sbuf(stw, nbw))

            # ---- layers ---------------------------------------------------
            edge_phase(xtab, msgp1, "a")
            nc.gpsimd.collective_compute(
                "ReduceScatter", add, ins=[msgp1[:, :]], outs=[msgs1[:, :]],
                replica_groups=rg)
            epilogue1(msgs1, n2s, w1t_sb, b1_sb if with_b else None)

            edge_phase(htab, msgp2, "b")
            nc.gpsimd.collective_compute(
                "ReduceScatter", add, ins=[msgp2[:, :]], outs=[msgs2[:, :]],
                replica_groups=rg)
            epilogue2(msgs2, w2t_sb, b2_sb if with_b else None)

    nc.compile()
    return nc


# ---------------------------------------------------------------------------
# Entry point
# ---------------------------------------------------------------------------

def _make_inmaps(inputs, meta, idx16, drel, with_b):
    S, SPAD, ncores = meta["S"], meta["SPAD"], meta["ncores"]
    n_nodes = meta["n_nodes"]
    x = np.ascontiguousarray(np.asarray(inputs["x"], np.float32))
    w1 = np.asarray(inputs["W1"], np.float32)
    w2 = np.asarray(inputs["W2"], np.float32)
    no1 = np.asarray(inputs["noise1"], np.float32)
    no2 = np.asarray(inputs["noise2"], np.float32)

    def shard(arr, c):
        lo = c * S
        hi = min(lo + S, n_nodes)
        out = np.zeros((SPAD, 128), np.float32)
        out[:hi - lo] = arr[lo:hi]
        return out

    import ml_dtypes
    ident = np.eye(128, dtype=np.float32)
    iotar = {}
    for _, _, gv in meta["runs"]:
        if gv not in iotar:
            iotar[gv] = np.ascontiguousarray(np.tile(
                (np.arange(gv * 128) // gv).astype(ml_dtypes.bfloat16),
                (128, 1)))
    in_maps = []
    for c in range(ncores):
        im = dict(
            xs=shard(x, c), n1s=shard(no1, c), n2s=shard(no2, c),
            w1t=np.ascontiguousarray(w1.T), w2t=np.ascontiguousarray(w2.T),
            idx=idx16[c], dstrel=drel[c], ident=ident,
            **{f"iotar{gv}": arr for gv, arr in iotar.items()},
        )
        if with_b:
            im["b1r"] = np.asarray(inputs["b1"], np.float32).reshape(1, 128)
            im["b2r"] = np.asarray(inputs["b2"], np.float32).reshape(1, 128)
        in_maps.append(im)
    return in_maps


def _run(inputs, ncores=NCORES, sim=False, trace=False):
    ei = np.asarray(inputs["edge_index"], np.int64)
    n_nodes = int(np.asarray(inputs["x"]).shape[0])
    meta, idx16, drel = _preprocess(ei[0], ei[1], n_nodes, ncores)
    with_b = bool(np.any(np.asarray(inputs["b1"])) or
                  np.any(np.asarray(inputs["b2"])))
    nc = _build_program(meta, with_b)
    in_maps = _make_inmaps(inputs, meta, idx16, drel, with_b)
    S = meta["S"]

    if sim:
        from concourse.bass_interp import MultiCoreSim
        msim = MultiCoreSim(nc, ncores, trace=trace)
        for c in range(ncores):
            for k, v in in_maps[c].items():
                msim.cores[c].tensor(k)[:] = v
        msim.simulate()
        results = [{"out": np.array(msim.cores[c].tensor("out"))}
                   for c in range(ncores)]
        res = msim
    else:
        res = run_bass_kernel_spmd(nc, in_maps, core_ids=list(range(ncores)),
                                   trace=trace)
        results = res.results

    parts = []
    for c in range(ncores):
        lo = c * S
        hi = min(lo + S, n_nodes)
        parts.append(results[c]["out"][:hi - lo])
    out = np.concatenate(parts, axis=0).astype(np.float32)
    return out, res


def kernel(**inputs) -> np.ndarray:
    out, _ = _run(inputs, ncores=NCORES, sim=False)
    return out
